# revision 1
# baseline (speedup 1.0000x reference)
"""Trainium2 Bass kernel for nn_MemResProjections (memory-residual attention).

Reference computation (B=4, S=2048, K=64, H=1024, fp32):
    normed = rmsnorm(hidden) * norm_w
    v_pool = concat([normed, memory], axis=1)            # (B, S+K, H)
    q = normed @ q_w.T ; k = v_pool @ k_w.T
    logits = q @ k.T / sqrt(H)  with causal mask on the local S block,
    memory columns fully visible
    attn = softmax(logits); h~ = attn @ v_pool
    alpha = sigmoid(hidden @ gate_w.T + gate_b)
    out = (1-alpha)*hidden + alpha*h~

Sharding: 8 cores = (batch b, half h) pairs; each core owns 1024 query rows.
Cores with h=1 see the first 1024 rows as a fully-visible "prefix"; cores with
h=0 get a zero prefix that is masked out via a per-core additive bias fused
into the exp() activation, keeping the SPMD program uniform.

Per-core dataflow (all matmuls in float32r = full-rate fp32, ~13-bit mantissa):
  A1: rmsnorm stats in natural layout; normed stripes PE-transposed into
      normedT (h on partitions); v = normed*norm_w spilled to DRAM scratch.
  A2: gate logits from normedT with the rstd factored out (gate uses raw x =
      normed * std, so scale the psum by std on eviction); sigmoid -> alpha
      spilled to DRAM.
  A3: qT = (q_w*norm_w).T-stationary @ normedT ; kT over own+prefix+memory.
  B:  scores^T tiles (t on partitions, 256 query cols) -> fused scale+mask+exp
      -> h~ accumulation (lhsT=exp^T) + denominator via ones-matmul; the
      [1,256] denominator hops to [128,2] natural layout with one SBUF DMA;
      output = x + alpha*(h~/den - x).
"""
import numpy as np

import concourse.bass as bass
import concourse.mybir as mybir
import concourse.tile as tile
from concourse.bass_utils import run_bass_kernel_spmd

F32 = mybir.dt.float32
F32R = mybir.dt.float32r
AFT = mybir.ActivationFunctionType

P = 128
H = 1024
S_OWN = 1024          # query rows per core
T_PREF = 1024         # prefix kv rows
T_MEM = 64            # memory kv rows
NJ = H // P           # h blocks
NS = S_OWN // P       # own stripes
SCALE = 1.0 / 32.0    # 1/sqrt(H)
EPS = 1e-6
NEG = -1.0e30

N_CORES = 8
B_FULL, S_FULL = 4, 2048


# ---------------------------------------------------------------- walrus fix
ENGINE_ATTR = {
    mybir.EngineType.PE: "tensor",
    mybir.EngineType.Activation: "scalar",
    mybir.EngineType.DVE: "vector",
    mybir.EngineType.Pool: "gpsimd",
    mybir.EngineType.SP: "sync",
}
DMA_OPS = ("InstDMACopy", "InstDMATranspose", "InstTensorLoad", "InstTensorSave",
           "InstCollectiveCompute")


def split_multi_waits(nc, limit=1, dma_limit=None):
    """This walrus build rejects engine instructions carrying more than one
    sem wait; hoist extras onto same-engine NOPs inserted just before."""
    n_split = 0
    for f in nc.m.functions:
        for blk in f.blocks:
            il = blk.instructions
            i = 0
            while i < len(il):
                ins = il[i]
                is_dma = type(ins).__name__ in DMA_OPS
                lim = dma_limit if is_dma else limit
                si = ins.sync_info
                waits = list(si.on_wait) if si is not None and si.on_wait else []
                if lim is not None and len(waits) > lim:
                    keep, extra = waits[:lim], waits[lim:]
                    si.on_wait.clear()
                    for w in keep:
                        si.on_wait.append(w)
                    eng = getattr(nc, ENGINE_ATTR[ins.engine])
                    for w in extra:
                        nop = eng.nop(nofuse=True, hint="wait_split")
                        nop.wait_op(bass.SemaphoreHandle(w.ant_name, w.id),
                                    w.wait_value, "sem-ge")
                        popped = nc.cur_bb.bb.instructions.pop()
                        assert popped.name == nop.ins.name
                        il.insert(i, nop.ins)
                        i += 1
                        n_split += 1
                i += 1
    return n_split


# ---------------------------------------------------------------- program
def build_nc():
    nc = bass.Bass()
    dp = lambda n, shp: nc.declare_dram_parameter(n, shp, F32, isOutput=False)
    x_own = dp("x_own", [S_OWN, H])
    x_pref = dp("x_pref", [T_PREF, H])
    mem = dp("mem", [T_MEM, H])
    memT = dp("memT", [H, T_MEM])
    qwT = dp("qwT", [H, H])        # (q_w * norm_w).T
    kwT = dp("kwT", [H, H])        # (k_w * norm_w).T
    gwT = dp("gwT", [H, H])        # gate_w.T
    w_bc = dp("w_bc", [P, H])      # norm_w broadcast
    b_bc = dp("b_bc", [P, H])      # gate_b broadcast
    pbias = dp("pbias", [P, 1])    # 0 (h=1) or -1e30 (h=0)
    onesc = dp("onesc", [P, 1])    # ones column (fp32r matmul operand)
    tri = dp("tri", [P, P])        # additive mask: 0 if col>=row else -1e30
    ident = dp("ident", [P, P])    # identity for PE transpose
    out = nc.declare_dram_parameter("out", [S_OWN, H], F32, isOutput=True)
    import os as _os
    DEBUG = _os.environ.get("DEBUG_KERNEL") == "1"
    if DEBUG:
        dbg_rden = nc.declare_dram_parameter("dbg_rden", [P, NS], F32,
                                             isOutput=True)

    v_dram = nc.dram_tensor("v_scratch", [S_OWN + T_PREF, H], F32)
    a_dram = nc.dram_tensor("alpha_scratch", [S_OWN, H], F32)

    T_ALL = S_OWN + T_PREF  # spill rows

    with tile.TileContext(nc) as tc:
        from contextlib import ExitStack
        with ExitStack() as ctx:
            # ---- long-lived pools
            const = ctx.enter_context(tc.tile_pool(name="const", bufs=1))
            proj = ctx.enter_context(tc.tile_pool(name="proj", bufs=1))

            eps_t = const.tile([P, 1], F32)
            nc.vector.memset(eps_t[:], EPS)
            ones_col = const.tile([P, 1], F32)
            nc.sync.dma_start(out=ones_col[:], in_=onesc[:])
            neg_t = const.tile([P, 1], F32)
            nc.vector.memset(neg_t[:], NEG)
            pb_t = const.tile([P, 1], F32)
            nc.sync.dma_start(out=pb_t[:], in_=pbias[:])
            tri_t = const.tile([P, P], F32)
            nc.sync.dma_start(out=tri_t[:], in_=tri[:])
            id_t = const.tile([P, P], F32R)
            nc.sync.dma_start(out=id_t[:], in_=ident[:].bitcast(F32R))
            std_all = const.tile([P, 16], F32)
            rstd_all = const.tile([P, 16], F32)
            rden = const.tile([P, NS], F32)
            memT_t = [const.tile([P, T_MEM], F32R, tag=f"memT{j}", name=f"memT{j}") for j in range(NJ)]
            for j in range(NJ):
                nc.sync.dma_start(out=memT_t[j][:],
                                  in_=memT[j * P:(j + 1) * P, :].bitcast(F32R))

            qT = [proj.tile([P, S_OWN], F32R, tag=f"qT{m}", name=f"qT{m}") for m in range(NJ)]
            kT = [proj.tile([P, S_OWN + T_PREF + T_MEM], F32R, tag=f"kT{m}", name=f"kT{m}")
                  for m in range(NJ)]

            # ================= phase A: norm, transpose, gate, projections
            with tc.tile_pool(name="aphase", bufs=1) as ap:
                normedT = [ap.tile([P, 2048], F32R, tag=f"nT{j}", name=f"nT{j}") for j in range(NJ)]

                # ---- A1: stats + normed + PE transpose + v spill
                with tc.tile_pool(name="a1s", bufs=2) as ast, \
                     tc.tile_pool(name="a1ps", bufs=4, space="PSUM") as aps:
                    w_bc_t = ast.tile([P, H], F32, bufs=1)
                    nc.sync.dma_start(out=w_bc_t[:], in_=w_bc[:])
                    sq = ast.tile([P, H], F32, bufs=1)  # shared Square scratch
                    for part in range(2):          # 0 = own, 1 = prefix
                        src = x_own if part == 0 else x_pref
                        for i in range(NS):
                            idx = part * NS + i
                            xt = ast.tile([P, H], F32, tag="xt")
                            nc.sync.dma_start(out=xt[:],
                                              in_=src[i * P:(i + 1) * P, :])
                            ss = ast.tile([P, 1], F32, tag="ss")
                            nc.scalar.activation(sq[:], xt[:], AFT.Square,
                                                 accum_out=ss[:])
                            nc.scalar.activation(std_all[:, idx:idx + 1], ss[:],
                                                 AFT.Sqrt, scale=1.0 / H,
                                                 bias=eps_t[:])
                            nc.vector.reciprocal(rstd_all[:, idx:idx + 1],
                                                 std_all[:, idx:idx + 1])
                            nrm = ast.tile([P, H], F32R, tag="nrm")
                            nc.scalar.activation(nrm[:], xt[:], AFT.Copy,
                                                 scale=rstd_all[:, idx:idx + 1])
                            # transpose 128x128 subtiles into normedT columns
                            for j in range(NJ):
                                tp = aps.tile([P, P], F32R, tag="tp")
                                nc.tensor.transpose(
                                    tp[:], nrm[:, j * P:(j + 1) * P], id_t[:])
                                nc.vector.tensor_copy(
                                    normedT[j][:, idx * P:(idx + 1) * P], tp[:])
                            # v = normed * norm_w (in place) -> spill
                            nc.vector.tensor_mul(nrm[:], nrm[:].bitcast(F32),
                                                 w_bc_t[:])
                            nc.sync.dma_start(
                                out=v_dram[idx * P:(idx + 1) * P, :],
                                in_=nrm[:].bitcast(F32))

                # ---- A2: gate -> alpha spill (gate = (normed @ gwT) * std)
                with tc.tile_pool(name="a2s", bufs=2) as gst, \
                     tc.tile_pool(name="a2ps", bufs=1, space="PSUM") as gps:
                    b_bc_t = gst.tile([P, H], F32, bufs=1)
                    nc.sync.dma_start(out=b_bc_t[:], in_=b_bc[:])
                    for oc in range(2):
                        pg = [gps.tile([P, 512], F32, tag=f"pg{si}",
                                       name=f"pg{si}") for si in range(NS)]
                        for j in range(NJ):
                            gwj = gst.tile([P, 512], F32R, tag="gwj")
                            nc.sync.dma_start(
                                out=gwj[:],
                                in_=gwT[j * P:(j + 1) * P,
                                        oc * 512:(oc + 1) * 512].bitcast(F32R))
                            for si in range(NS):
                                nc.tensor.matmul(
                                    pg[si][:],
                                    normedT[j][:, si * P:(si + 1) * P],
                                    gwj[:],
                                    start=(j == 0), stop=(j == NJ - 1))
                        for si in range(NS):
                            gl = gst.tile([P, 512], F32, tag="gl")
                            nc.scalar.activation(gl[:], pg[si][:], AFT.Copy,
                                                 scale=std_all[:, si:si + 1])
                            nc.vector.tensor_add(
                                gl[:], gl[:], b_bc_t[:, oc * 512:(oc + 1) * 512])
                            al = gst.tile([P, 512], F32, tag="al")
                            nc.scalar.activation(al[:], gl[:], AFT.Sigmoid)
                            nc.sync.dma_start(
                                out=a_dram[si * P:(si + 1) * P,
                                           oc * 512:(oc + 1) * 512],
                                in_=al[:])

                # ---- A3: qT / kT projections
                with tc.tile_pool(name="wstrip", bufs=2) as wsp, \
                     tc.tile_pool(name="a3ps", bufs=2, space="PSUM") as aps3:
                    for m in range(NJ):
                        qs = wsp.tile([P, H], F32R, tag="qs")
                        ks = wsp.tile([P, H], F32R, tag="ks")
                        for j in range(NJ):
                            nc.sync.dma_start(
                                out=qs[:, j * P:(j + 1) * P],
                                in_=qwT[j * P:(j + 1) * P,
                                        m * P:(m + 1) * P].bitcast(F32R))
                            nc.sync.dma_start(
                                out=ks[:, j * P:(j + 1) * P],
                                in_=kwT[j * P:(j + 1) * P,
                                        m * P:(m + 1) * P].bitcast(F32R))
                        # qT_m over own cols
                        for sc in range(2):
                            pq = aps3.tile([P, 512], F32, tag="pq")
                            for j in range(NJ):
                                nc.tensor.matmul(
                                    pq[:], qs[:, j * P:(j + 1) * P],
                                    normedT[j][:, sc * 512:(sc + 1) * 512],
                                    start=(j == 0), stop=(j == NJ - 1))
                            nc.vector.tensor_copy(
                                qT[m][:, sc * 512:(sc + 1) * 512], pq[:])
                        # kT_m over own+prefix cols
                        for sc in range(4):
                            pk = aps3.tile([P, 512], F32, tag="pq")
                            for j in range(NJ):
                                nc.tensor.matmul(
                                    pk[:], ks[:, j * P:(j + 1) * P],
                                    normedT[j][:, sc * 512:(sc + 1) * 512],
                                    start=(j == 0), stop=(j == NJ - 1))
                            nc.vector.tensor_copy(
                                kT[m][:, sc * 512:(sc + 1) * 512], pk[:])
                        # kT_m over memory cols
                        pkm = aps3.tile([P, T_MEM], F32, tag="pkm")
                        for j in range(NJ):
                            nc.tensor.matmul(pkm[:], ks[:, j * P:(j + 1) * P],
                                             memT_t[j][:],
                                             start=(j == 0), stop=(j == NJ - 1))
                        nc.vector.tensor_copy(kT[m][:, 2048:2048 + T_MEM], pkm[:])

            # ================= phase B: attention
            with tc.tile_pool(name="bres", bufs=1) as bres, \
                 tc.tile_pool(name="bstream", bufs=2) as bst, \
                 tc.tile_pool(name="bexp", bufs=3) as bexp, \
                 tc.tile_pool(name="bps", bufs=2, space="PSUM") as bps, \
                 tc.tile_pool(name="bph", bufs=1, space="PSUM") as bph:
                vpref = [bres.tile([P, H], F32R, tag=f"vp{t}", name=f"vp{t}") for t in range(8)]
                for t in range(8):
                    nc.sync.dma_start(
                        out=vpref[t][:],
                        in_=v_dram[S_OWN + t * P:S_OWN + (t + 1) * P, :]
                            .bitcast(F32R))
                vmem = bres.tile([T_MEM, H], F32R)
                nc.sync.dma_start(out=vmem[:], in_=mem[:].bitcast(F32R))

                NHG = 4  # half-groups of 2 stripes (256 query cols)
                for hg in range(NHG):
                    s0 = hg * 256
                    # tau blocks: (kind, index): own 0..2hg+1, prefix 0..7, mem
                    taus = ([("own", t) for t in range(2 * hg + 2)]
                            + [("pref", t) for t in range(8)]
                            + [("mem", 0)])
                    ph = {(sl, hc): bph.tile([P, 512], F32, tag=f"ph{sl}{hc}", name=f"ph{sl}{hc}")
                          for sl in range(2) for hc in range(2)}
                    pd = [bph.tile([P, 1], F32, tag=f"pd{sl}", name=f"pd{sl}")
                          for sl in range(2)]
                    for ti, (kind, t) in enumerate(taus):
                        first, last = ti == 0, ti == len(taus) - 1
                        rows = T_MEM if kind == "mem" else P
                        # scores^T [rows, 256]
                        ps = bps.tile([P, 256], F32, tag="ps")
                        if kind == "own":
                            koff = t * P
                        elif kind == "pref":
                            koff = S_OWN + t * P
                        else:
                            koff = 2048
                        for m in range(NJ):
                            nc.tensor.matmul(
                                ps[:rows, :], kT[m][:, koff:koff + rows],
                                qT[m][:, s0:s0 + 256],
                                start=(m == 0), stop=(m == NJ - 1))
                        # exp with fused scale (+mask / prefix bias)
                        et = bexp.tile([P, 256], F32R, tag="et")
                        if kind == "own":
                            sl_d = t - 2 * hg
                            if sl_d == 0:
                                nc.vector.tensor_add(ps[:, 0:P], ps[:, 0:P],
                                                     tri_t[:])
                                nc.scalar.activation(et[:], ps[:], AFT.Exp,
                                                     scale=SCALE)
                            elif sl_d == 1:
                                nc.scalar.activation(et[:, 0:P], ps[:, 0:P],
                                                     AFT.Exp, scale=SCALE,
                                                     bias=neg_t[:])
                                nc.vector.tensor_add(ps[:, P:256], ps[:, P:256],
                                                     tri_t[:])
                                nc.scalar.activation(et[:, P:256], ps[:, P:256],
                                                     AFT.Exp, scale=SCALE)
                            else:
                                nc.scalar.activation(et[:], ps[:], AFT.Exp,
                                                     scale=SCALE)
                        elif kind == "pref":
                            nc.scalar.activation(et[:], ps[:], AFT.Exp,
                                                 scale=SCALE, bias=pb_t[:])
                        else:
                            nc.scalar.activation(et[:rows, :], ps[:rows, :],
                                                 AFT.Exp, scale=SCALE)
                        # v tile
                        if kind == "own":
                            vt = bst.tile([P, H], F32R, tag="vb", bufs=3)
                            nc.sync.dma_start(
                                out=vt[:],
                                in_=v_dram[t * P:(t + 1) * P, :].bitcast(F32R))
                        elif kind == "pref":
                            vt = vpref[t]
                        else:
                            vt = vmem
                        # h~ accumulation + denominator (same stationary et)
                        for sl in range(2):
                            for hc in range(2):
                                nc.tensor.matmul(
                                    ph[(sl, hc)][:],
                                    et[:rows, sl * P:(sl + 1) * P],
                                    vt[:rows, hc * 512:(hc + 1) * 512],
                                    start=first, stop=last,
                                    skip_group_check=True)
                            nc.tensor.matmul(
                                pd[sl][:],
                                et[:rows, sl * P:(sl + 1) * P].bitcast(F32),
                                ones_col[:rows, :],
                                start=first, stop=last,
                                skip_group_check=True)
                    for sl in range(2):
                        sidx = 2 * hg + sl
                        nc.vector.reciprocal(rden[:, sidx:sidx + 1], pd[sl][:])
                    # evict h~, final combine
                    for sl in range(2):
                        sidx = 2 * hg + sl
                        hsb = bst.tile([P, H], F32, tag="hsb")
                        for hc in range(2):
                            nc.scalar.activation(
                                hsb[:, hc * 512:(hc + 1) * 512], ph[(sl, hc)][:],
                                AFT.Copy, scale=rden[:, sidx:sidx + 1])
                        xs = bst.tile([P, H], F32, tag="xs")
                        nc.sync.dma_start(out=xs[:],
                                          in_=x_own[sidx * P:(sidx + 1) * P, :])
                        als = bst.tile([P, H], F32, tag="als")
                        nc.sync.dma_start(out=als[:],
                                          in_=a_dram[sidx * P:(sidx + 1) * P, :])
                        nc.vector.tensor_sub(hsb[:], hsb[:], xs[:])
                        nc.vector.tensor_mul(hsb[:], hsb[:], als[:])
                        nc.vector.tensor_add(hsb[:], hsb[:], xs[:])
                        nc.sync.dma_start(out=out[sidx * P:(sidx + 1) * P, :],
                                          in_=hsb[:])
                if DEBUG:
                    nc.sync.dma_start(out=dbg_rden[:], in_=rden[:])

    import os
    if os.environ.get("NO_WAIT_SPLIT") != "1":
        split_multi_waits(nc, limit=1, dma_limit=1)
    return nc


_NC_CACHE = None
_LAST_IN_MAPS = None


def _get_nc():
    global _NC_CACHE
    if _NC_CACHE is None:
        _NC_CACHE = build_nc()
    return _NC_CACHE


def prepare_in_maps(hidden_states, memory_state, q_w, k_w, norm_w, gate_w,
                    gate_b):
    hidden_states = np.asarray(hidden_states, dtype=np.float32)
    memory_state = np.asarray(memory_state, dtype=np.float32)
    q_w = np.asarray(q_w, dtype=np.float32)
    k_w = np.asarray(k_w, dtype=np.float32)
    norm_w = np.asarray(norm_w, dtype=np.float32)
    gate_w = np.asarray(gate_w, dtype=np.float32)
    gate_b = np.asarray(gate_b, dtype=np.float32)

    qwT = np.ascontiguousarray((q_w * norm_w[None, :]).T)
    kwT = np.ascontiguousarray((k_w * norm_w[None, :]).T)
    gwT = np.ascontiguousarray(gate_w.T)
    w_bc = np.ascontiguousarray(np.broadcast_to(norm_w, (P, H)))
    b_bc = np.ascontiguousarray(np.broadcast_to(gate_b, (P, H)))
    tri = np.where(np.arange(P)[None, :] >= np.arange(P)[:, None],
                   np.float32(0.0), np.float32(NEG)).astype(np.float32)
    ident = np.eye(P, dtype=np.float32)
    zeros_pref = np.zeros((T_PREF, H), dtype=np.float32)

    in_maps = []
    for c in range(N_CORES):
        b, h = divmod(c, 2)
        x_own = np.ascontiguousarray(hidden_states[b, h * S_OWN:(h + 1) * S_OWN])
        x_pref = (np.ascontiguousarray(hidden_states[b, :T_PREF]) if h == 1
                  else zeros_pref)
        memb = np.ascontiguousarray(memory_state[b])
        in_maps.append({
            "x_own": x_own,
            "x_pref": x_pref,
            "mem": memb,
            "memT": np.ascontiguousarray(memb.T),
            "qwT": qwT, "kwT": kwT, "gwT": gwT,
            "w_bc": w_bc, "b_bc": b_bc,
            "pbias": np.full((P, 1), 0.0 if h == 1 else NEG, np.float32),
            "onesc": np.ones((P, 1), np.float32),
            "tri": tri, "ident": ident,
        })
    return in_maps


def kernel(**inputs):
    in_maps = prepare_in_maps(**inputs)
    global _LAST_IN_MAPS
    _LAST_IN_MAPS = in_maps
    nc = _get_nc()
    res = run_bass_kernel_spmd(nc, in_maps, list(range(N_CORES)))
    out = np.empty((B_FULL, S_FULL, H), dtype=np.float32)
    for c in range(N_CORES):
        b, h = divmod(c, 2)
        out[b, h * S_OWN:(h + 1) * S_OWN] = res.results[c]["out"]
    return out



# revision 19
# speedup vs baseline: 1.6819x; 1.6819x over previous
"""Trainium2 Bass kernel for nn_MemResProjections (memory-residual attention).

Reference computation (B=4, S=2048, K=64, H=1024, fp32):
    normed = rmsnorm(hidden) * norm_w
    v_pool = concat([normed, memory], axis=1)            # (B, S+K, H)
    q = normed @ q_w.T ; k = v_pool @ k_w.T
    logits = q @ k.T / sqrt(H)  with causal mask on the local S block,
    memory columns fully visible
    attn = softmax(logits); h~ = attn @ v_pool
    alpha = sigmoid(hidden @ gate_w.T + gate_b)
    out = (1-alpha)*hidden + alpha*h~

Sharding: 8 cores = (batch b, parity h) pairs.  Core (b,h) owns the 8
query blocks {h, h+2, ..., h+14} (128 rows each) of batch b.  KV blocks are
laid out per-core in a "local" interleaved order  [own_0, oth_0, own_1,
oth_1, ...]  so that own query block i needs exactly the kv-local prefix
0..2i+1 on every core -- one uniform program, per-core data.  The single
per-core mask difference (is the interleaved neighbour before or after me?)
is a data tile (nxt2: -1e30 for h=0, 0 for h=1).

All activations/weights move as bf16 (host-prepped, incl. transposed copies
of x so no PE transposes are needed); psum accumulation fp32; rmsnorm stats
on-device in fp32.  No DRAM spills: v, kT, qT, alpha stay SBUF-resident.
rstd is folded in at PSUM eviction via a partition-broadcast tile built
with a tiny ones-matmul.  Phase order keeps PE hot: stats (ACT) -> kT ->
gate -> qT -> v scaling -> attention; weight/xT loads ride the scalar
HWDGE ring so the sync ring serves the x stripes immediately.
"""
import numpy as np
import ml_dtypes

import concourse.bass as bass
import concourse.mybir as mybir
import concourse.tile as tile
from concourse.bass_utils import run_bass_kernel_spmd

BF16 = mybir.dt.bfloat16
F32 = mybir.dt.float32
F32R = mybir.dt.float32r
AFT = mybir.ActivationFunctionType
NPBF16 = ml_dtypes.bfloat16

P = 128
H = 1024
NJ = H // P           # 8 h-blocks
NKV = 16              # local kv blocks (2048 rows)
T_MEM = 64
SCALE = 1.0 / 32.0    # 1/sqrt(H)
EPS = 1e-6
NEG = -1.0e30

N_CORES = 8
B_FULL, S_FULL = 4, 2048


# ---------------------------------------------------------------- walrus fix
ENGINE_ATTR = {
    mybir.EngineType.PE: "tensor",
    mybir.EngineType.Activation: "scalar",
    mybir.EngineType.DVE: "vector",
    mybir.EngineType.Pool: "gpsimd",
    mybir.EngineType.SP: "sync",
}
DMA_OPS = ("InstDMACopy", "InstDMATranspose", "InstTensorLoad", "InstTensorSave",
           "InstCollectiveCompute")


def split_multi_waits(nc, limit=1, dma_limit=None):
    """This walrus build rejects engine instructions carrying more than one
    sem wait; hoist extras onto same-engine NOPs inserted just before."""
    n_split = 0
    for f in nc.m.functions:
        for blk in f.blocks:
            il = blk.instructions
            i = 0
            while i < len(il):
                ins = il[i]
                is_dma = type(ins).__name__ in DMA_OPS
                lim = dma_limit if is_dma else limit
                si = ins.sync_info
                waits = list(si.on_wait) if si is not None and si.on_wait else []
                if lim is not None and len(waits) > lim:
                    keep, extra = waits[:lim], waits[lim:]
                    si.on_wait.clear()
                    for w in keep:
                        si.on_wait.append(w)
                    eng = getattr(nc, ENGINE_ATTR[ins.engine])
                    for w in extra:
                        nop = eng.nop(nofuse=True, hint="wait_split")
                        nop.wait_op(bass.SemaphoreHandle(w.ant_name, w.id),
                                    w.wait_value, "sem-ge")
                        popped = nc.cur_bb.bb.instructions.pop()
                        assert popped.name == nop.ins.name
                        il.insert(i, nop.ins)
                        i += 1
                        n_split += 1
                i += 1
    return n_split


# ---------------------------------------------------------------- program
def build_nc():
    nc = bass.Bass()
    dp = lambda n, shp, dt: nc.declare_dram_parameter(n, shp, dt, isOutput=False)
    x_lk = dp("x_lk", [2048, H], BF16)        # natural, local-kv row order
    xT_lk = dp("xT_lk", [H, 2048], BF16)      # transposed, same col order
    xT_own = dp("xT_own", [H, 1024], BF16)    # own columns only
    mem = dp("mem", [T_MEM, H], BF16)
    memT_slab = dp("memT_slab", [P, 512], BF16)   # [p, 64j+t] = mem[t, 128j+p]
    qw_slab = dp("qw_slab", [H, H], BF16)     # slab m rows: [p, 128j+c] = qwT_w[128j+p, 128m+c]
    kw_slab = dp("kw_slab", [H, H], BF16)
    gw = dp("gw", [H, H], BF16)               # gate_w.T  [h, o]
    b_row = dp("b_row", [1, H], BF16)         # gate_b
    w_bc = dp("w_bc", [P, H], BF16)           # norm_w broadcast
    tri2 = dp("tri2", [P, 256], F32)          # [tri | 0]
    nxt2 = dp("nxt2", [P, 256], F32)          # [nextb | 0], nextb = -1e30 (h=0) / 0 (h=1)
    onesc = dp("onesc", [P, 1], BF16)
    onesr_b = dp("onesr_b", [1, P], BF16)
    onesr_f = dp("onesr_f", [1, P], F32)
    out = nc.declare_dram_parameter("out", [1024, H], BF16, isOutput=True)
    import os as _os
    DEBUG = _os.environ.get("DEBUG_KERNEL") == "1"
    if DEBUG:
        dout = lambda n, shp: nc.declare_dram_parameter(n, shp, F32,
                                                        isOutput=True)
        dbg_rstd = dout("dbg_rstd", [P, NKV])
        dbg_bc = dout("dbg_bc", [P, 2048])
        dbg_al0 = dout("dbg_al0", [P, H])
        dbg_kt0 = nc.declare_dram_parameter("dbg_kt0", [P, 2048 + T_MEM],
                                            BF16, isOutput=True)
        dbg_qt0 = nc.declare_dram_parameter("dbg_qt0", [P, 1024], BF16,
                                            isOutput=True)
        dbg_v0 = nc.declare_dram_parameter("dbg_v0", [P, H], BF16,
                                           isOutput=True)
        dbg_et0 = nc.declare_dram_parameter("dbg_et0", [P, 256], BF16,
                                            isOutput=True)
        dbg_etm = nc.declare_dram_parameter("dbg_etm", [P, 256], BF16,
                                            isOutput=True)
        dbg_den = dout("dbg_den", [P, 2])
        dbg_h00 = dout("dbg_h00", [P, 512])

    with tile.TileContext(nc) as tc:
        from contextlib import ExitStack
        with ExitStack() as ctx:
            # ---- long-lived pools (v/qT open after phase A to keep the
            # peak under the SBUF cap -- pools reserve for their whole scope)
            const = ctx.enter_context(tc.tile_pool(name="const", bufs=1))
            ktp = ctx.enter_context(tc.tile_pool(name="ktp", bufs=1))
            apool = ctx.enter_context(tc.tile_pool(name="apool", bufs=1))

            eps_t = const.tile([P, 1], F32)
            nc.vector.memset(eps_t[:], EPS)
            rstd_nat = const.tile([P, NKV], F32)
            sdt_all = const.tile([P, NKV], F32)
            scl_nat = const.tile([P, NKV], F32)
            rrow_own = const.tile([1, 1024], F32R)
            std_row = const.tile([1, 1024], F32)
            rrow_f = const.tile([1, 1024], F32)
            rstd_bc_own = const.tile([P, 1024], F32)

            kT = [ktp.tile([P, 2048 + T_MEM], BF16, tag=f"kT{m}", name=f"kT{m}")
                  for m in range(NJ)]
            alpha = [apool.tile([P, H], BF16, tag=f"al{i}", name=f"al{i}")
                     for i in range(8)]

            # xT_lk tiles live through kT + gate.  Only the 8 xT loads ride
            # the scalar HWDGE ring (so ACT is free after ~5us); everything
            # else is issued on the sync ring in criticality order.
            xop = ctx.enter_context(tc.tile_pool(name="xop", bufs=1))
            with tc.tile_pool(name="xtpool", bufs=1) as xtpool, \
                 tc.tile_pool(name="sqo", bufs=1) as sqo, \
                 tc.tile_pool(name="kwp", bufs=1) as kwp, \
                 tc.tile_pool(name="gwp", bufs=1) as gwp:
                xT_t = [xtpool.tile([P, 2048], BF16, tag=f"xT{j}", name=f"xT{j}")
                        for j in range(NJ)]
                for j in range(NJ):
                    nc.scalar.dma_start(out=xT_t[j][:],
                                        in_=xT_lk[j * P:(j + 1) * P, :])
                memT_t = const.tile([P, 512], BF16)
                nc.sync.dma_start(out=memT_t[:], in_=memT_slab[:])
                onesc_t = const.tile([P, 1], BF16)
                nc.sync.dma_start(out=onesc_t[:], in_=onesc[:])
                onesrf_t = const.tile([1, P], F32R)
                nc.sync.dma_start(out=onesrf_t[:], in_=onesr_f[:].bitcast(F32R))
                kw_t = [kwp.tile([P, H], BF16, tag=f"kw{m}", name=f"kw{m}")
                        for m in range(NJ)]
                for m in range(NJ):
                    nc.sync.dma_start(out=kw_t[m][:],
                                      in_=kw_slab[m * P:(m + 1) * P, :])

                # ---- A1: rmsnorm stats on ACT only (reciprocals deferred so
                # the DVE stream is free for kT evictions)
                with tc.tile_pool(name="a1s", bufs=3) as a1s:
                    for l in range(NKV):
                        xt = a1s.tile([P, H], BF16, tag="xt")
                        nc.sync.dma_start(out=xt[:],
                                          in_=x_lk[l * P:(l + 1) * P, :])
                        sq = a1s.tile([P, H], BF16, tag="sq", bufs=2)
                        ss = a1s.tile([P, 1], F32, tag="ss")
                        nc.scalar.activation(sq[:], xt[:], AFT.Square,
                                             accum_out=ss[:])
                        nc.scalar.activation(sdt_all[:, l:l + 1], ss[:],
                                             AFT.Sqrt, scale=1.0 / H,
                                             bias=eps_t[:])
                xo_t = [xop.tile([P, 1024], BF16, tag=f"xo{j}", name=f"xo{j}")
                        for j in range(NJ)]
                for j in range(NJ):
                    nc.sync.dma_start(out=xo_t[j][:],
                                      in_=xT_own[j * P:(j + 1) * P, :])
                gw_t = [gwp.tile([P, H], BF16, tag=f"gw{j}", name=f"gw{j}")
                        for j in range(NJ)]
                for j in range(NJ):
                    nc.sync.dma_start(out=gw_t[j][:],
                                      in_=gw[j * P:(j + 1) * P, :])
                onesrb_t = const.tile([1, P], BF16)
                nc.sync.dma_start(out=onesrb_t[:], in_=onesr_b[:])
                b_row_t = const.tile([1, H], BF16)
                nc.sync.dma_start(out=b_row_t[:], in_=b_row[:])
                # squares of own transposed cols for the row-form stats
                sq_own = [sqo.tile([P, 1024], BF16, tag=f"sqo{j}", name=f"sqo{j}")
                          for j in range(NJ)]
                for j in range(NJ):
                    nc.scalar.activation(sq_own[j][:], xo_t[j][:],
                                         AFT.Square)

                # ---- kT projection (row-form q-rstd stats interleaved so
                # rstd_bc_own is ready long before the qT evictions)
                with tc.tile_pool(name="rowp", bufs=1, space="PSUM") as rowp, \
                     tc.tile_pool(name="pps", bufs=4, space="PSUM") as pps:
                    for m in range(NJ):
                        kwm = kw_t[m]
                        if m == 3:
                            for c in range(2):
                                pv = rowp.tile([1, 512], F32, tag="pv")
                                for j in range(NJ):
                                    nc.tensor.matmul(
                                        pv[:], onesc_t[:],
                                        sq_own[j][:, c * 512:(c + 1) * 512],
                                        start=(j == 0), stop=(j == NJ - 1))
                                nc.scalar.activation(
                                    std_row[0:1, c * 512:(c + 1) * 512],
                                    pv[:], AFT.Sqrt, scale=1.0 / H,
                                    bias=eps_t[0:1, :])
                            nc.vector.reciprocal(rrow_f[:], std_row[:])
                            nc.scalar.activation(rrow_own[:], rrow_f[:],
                                                 AFT.Copy)
                        if m == 4:
                            for c in range(2):
                                pb = rowp.tile([P, 512], F32, tag="pbc")
                                nc.tensor.matmul(
                                    pb[:], onesrf_t[:],
                                    rrow_own[0:1, c * 512:(c + 1) * 512],
                                    start=True, stop=True)
                                nc.vector.tensor_copy(
                                    rstd_bc_own[:, c * 512:(c + 1) * 512],
                                    pb[:])
                        for c in range(4):
                            pk = pps.tile([P, 512], F32, tag="pk")
                            for j in range(NJ):
                                nc.tensor.matmul(
                                    pk[:], kwm[:, j * P:(j + 1) * P],
                                    xT_t[j][:, c * 512:(c + 1) * 512],
                                    start=(j == 0), stop=(j == NJ - 1))
                            nc.vector.tensor_copy(
                                kT[m][:, c * 512:(c + 1) * 512], pk[:])
                        pkm = pps.tile([P, T_MEM], F32, tag="pkm", bufs=2)
                        for j in range(NJ):
                            nc.tensor.matmul(
                                pkm[:], kwm[:, j * P:(j + 1) * P],
                                memT_t[:, j * T_MEM:(j + 1) * T_MEM],
                                start=(j == 0), stop=(j == NJ - 1))
                        nc.vector.tensor_copy(kT[m][:, 2048:2048 + T_MEM],
                                              pkm[:])

                # ---- gate (uses xT_lk as lhsT; alpha stored bf16)
                with tc.tile_pool(name="gps", bufs=2, space="PSUM") as gps:
                    for i in range(8):
                        pg = [gps.tile([P, 512], F32, tag=f"pg{oc}",
                                       name=f"pg{i}_{oc}")
                              for oc in range(2)]
                        for j in range(NJ):
                            for oc in range(2):
                                nc.tensor.matmul(
                                    pg[oc][:],
                                    xT_t[j][:, 2 * i * P:(2 * i + 1) * P],
                                    gw_t[j][:, oc * 512:(oc + 1) * 512],
                                    start=(j == 0), stop=False)
                        for oc in range(2):
                            nc.tensor.matmul(
                                pg[oc][:], onesrb_t[:],
                                b_row_t[0:1, oc * 512:(oc + 1) * 512],
                                start=False, stop=True)
                            nc.scalar.activation(
                                alpha[i][:, oc * 512:(oc + 1) * 512],
                                pg[oc][:], AFT.Sigmoid)

            vpool = ctx.enter_context(tc.tile_pool(name="vpool", bufs=1))
            qtp = ctx.enter_context(tc.tile_pool(name="qtp", bufs=1))
            v = [vpool.tile([P, H], BF16, tag=f"v{l}", name=f"v{l}")
                 for l in range(NKV)]
            qT = [qtp.tile([P, 1024], BF16, tag=f"qT{m}", name=f"qT{m}")
                  for m in range(NJ)]

            # ---- qT projection (xT_lk released; own transposed cols)
            with tc.tile_pool(name="wsl2", bufs=2) as wsl2, \
                 tc.tile_pool(name="pps2", bufs=4, space="PSUM") as pps2:
                for l in range(NKV):
                    nc.vector.reciprocal(rstd_nat[:, l:l + 1],
                                         sdt_all[:, l:l + 1])
                nc.vector.tensor_scalar_mul(scl_nat[:], rstd_nat[:], SCALE)
                for m in range(NJ):
                    qwm = wsl2.tile([P, H], BF16, tag="qwm")
                    nc.scalar.dma_start(out=qwm[:],
                                        in_=qw_slab[m * P:(m + 1) * P, :])
                    for c in range(2):
                        pq = pps2.tile([P, 512], F32, tag="pq")
                        for j in range(NJ):
                            nc.tensor.matmul(
                                pq[:], qwm[:, j * P:(j + 1) * P],
                                xo_t[j][:, c * 512:(c + 1) * 512],
                                start=(j == 0), stop=(j == NJ - 1))
                        nc.vector.tensor_mul(
                            qT[m][:, c * 512:(c + 1) * 512], pq[:],
                            rstd_bc_own[:, c * 512:(c + 1) * 512])

            # ---- v pass: v = x * rstd * w  (x re-streamed)
            with tc.tile_pool(name="vps", bufs=3) as vps:
                w_bc_t = const.tile([P, H], BF16)
                nc.sync.dma_start(out=w_bc_t[:], in_=w_bc[:])
                v_mem = const.tile([T_MEM, H], BF16)
                nc.sync.dma_start(out=v_mem[:], in_=mem[:])
                tri2_t = const.tile([P, 256], F32)
                nc.sync.dma_start(out=tri2_t[:], in_=tri2[:])
                nxt2_t = const.tile([P, 256], F32)
                nc.sync.dma_start(out=nxt2_t[:], in_=nxt2[:])
                for l in range(NKV):
                    x2 = vps.tile([P, H], BF16, tag="x2")
                    nc.sync.dma_start(out=x2[:],
                                      in_=x_lk[l * P:(l + 1) * P, :])
                    nc.scalar.activation(v[l][:], x2[:], AFT.Copy,
                                         scale=rstd_nat[:, l:l + 1])
                    nc.vector.tensor_mul(v[l][:], v[l][:], w_bc_t[:])

            if DEBUG:
                nc.sync.dma_start(out=dbg_rstd[:], in_=rstd_nat[:])
                nc.sync.dma_start(
                    out=dbg_bc[:, 0:1024], in_=rstd_bc_own[:])
                nc.sync.dma_start(
                    out=dbg_bc[:, 1024:1040], in_=rstd_nat[:])
                al0f = const.tile([P, H], F32)
                nc.vector.tensor_copy(al0f[:], alpha[0][:])
                nc.sync.dma_start(out=dbg_al0[:], in_=al0f[:])
                nc.sync.dma_start(out=dbg_kt0[:], in_=kT[0][:])
                nc.sync.dma_start(out=dbg_qt0[:], in_=qT[0][:])
                nc.sync.dma_start(out=dbg_v0[:], in_=v[0][:])

            # ================= phase B: attention (4 balanced pairs)
            with tc.tile_pool(name="bst", bufs=2) as bst, \
                 tc.tile_pool(name="bet", bufs=4) as bet, \
                 tc.tile_pool(name="bps", bufs=2, space="PSUM") as bps, \
                 tc.tile_pool(name="bph", bufs=1, space="PSUM") as bph, \
                 tc.tile_pool(name="bpd", bufs=1, space="PSUM") as bpd:
                for p in range(4):
                    q0 = 256 * p
                    xs = []
                    for sl in range(2):
                        xsb = bst.tile([P, H], BF16, tag=f"xsb{sl}")
                        i = 2 * p + sl
                        nc.sync.dma_start(
                            out=xsb[:],
                            in_=x_lk[2 * i * P:(2 * i + 1) * P, :])
                        xst = bst.tile([P, H], F32, tag=f"xs{sl}")
                        nc.vector.tensor_copy(xst[:], xsb[:])
                        xs.append(xst)
                    ph = {(sl, hc): bph.tile([P, 512], F32, tag=f"ph{sl}{hc}",
                                             name=f"ph{sl}{hc}")
                          for sl in range(2) for hc in range(2)}
                    # NB: separate tiles => separate PSUM banks.  start=True
                    # zeroes a whole 2KB zero region, so the two q-blocks'
                    # denominators must not share a bank.
                    pd = [bpd.tile([P, 1], F32, tag=f"pd{sl}", name=f"pd{sl}")
                          for sl in range(2)]

                    # tasks: ("sh", t) both q-blocks; ("so", t) second only; mem
                    tasks = ([("sh", t) for t in range(4 * p + 2)]
                             + [("so", 4 * p + 2), ("so", 4 * p + 3)]
                             + [("mem", 0)])

                    def emit_scores(task):
                        kind, t = task
                        if kind == "sh":
                            ps = bps.tile([P, 256], F32, tag="ps")
                            for m in range(NJ):
                                nc.tensor.matmul(
                                    ps[:], kT[m][:, t * P:(t + 1) * P],
                                    qT[m][:, q0:q0 + 256],
                                    start=(m == 0), stop=(m == NJ - 1))
                            if t == 4 * p:
                                nc.vector.tensor_add(ps[:], ps[:], tri2_t[:])
                            elif t == 4 * p + 1:
                                nc.vector.tensor_add(ps[:], ps[:], nxt2_t[:])
                            et = bet.tile([P, 256], BF16, tag="et")
                            nc.scalar.activation(et[:], ps[:], AFT.Exp,
                                                 scale=scl_nat[:, t:t + 1])
                            return (kind, t, et, P)
                        if kind == "so":
                            ps = bps.tile([P, 256], F32, tag="ps")
                            for m in range(NJ):
                                nc.tensor.matmul(
                                    ps[:, 0:P], kT[m][:, t * P:(t + 1) * P],
                                    qT[m][:, q0 + P:q0 + 256],
                                    start=(m == 0), stop=(m == NJ - 1))
                            bias = tri2_t if t == 4 * p + 2 else nxt2_t
                            nc.vector.tensor_add(ps[:, 0:P], ps[:, 0:P],
                                                 bias[:, 0:P])
                            et = bet.tile([P, 256], BF16, tag="et")
                            nc.scalar.activation(et[:, 0:P], ps[:, 0:P],
                                                 AFT.Exp,
                                                 scale=scl_nat[:, t:t + 1])
                            return (kind, t, et, P)
                        # mem
                        ps = bps.tile([P, 256], F32, tag="ps")
                        for m in range(NJ):
                            nc.tensor.matmul(
                                ps[:T_MEM, :], kT[m][:, 2048:2048 + T_MEM],
                                qT[m][:, q0:q0 + 256],
                                start=(m == 0), stop=(m == NJ - 1))
                        et = bet.tile([P, 256], BF16, tag="et")
                        nc.scalar.activation(et[:T_MEM, :], ps[:T_MEM, :],
                                             AFT.Exp, scale=SCALE)
                        return (kind, t, et, T_MEM)

                    def emit_hv(sc, first, last):
                        kind, t, et, rows = sc
                        if kind == "sh":
                            vt, sls = v[t], (0, 1)
                        elif kind == "so":
                            vt, sls = v[t], (1,)
                        else:
                            vt, sls = v_mem, (0, 1)
                        for sl in sls:
                            if kind == "so":
                                lhs = et[:rows, 0:P]
                            else:
                                lhs = et[:rows, sl * P:(sl + 1) * P]
                            for hc in range(2):
                                nc.tensor.matmul(
                                    ph[(sl, hc)][:], lhs,
                                    vt[:rows, hc * 512:(hc + 1) * 512],
                                    start=first, stop=last,
                                    skip_group_check=True)
                            nc.tensor.matmul(
                                pd[sl][:], lhs, onesc_t[:rows, :],
                                start=first, stop=last,
                                skip_group_check=True)

                    prev = None
                    for ti, task in enumerate(tasks):
                        sc = emit_scores(task)
                        if DEBUG and p == 0 and ti == 0:
                            nc.sync.dma_start(out=dbg_et0[:], in_=sc[2][:])
                        if DEBUG and p == 0 and task[0] == "mem":
                            nc.sync.dma_start(out=dbg_etm[:], in_=sc[2][:])
                        if prev is not None:
                            emit_hv(prev, first=(prev[1] == 0 and prev[0] == "sh"),
                                    last=False)
                        prev = sc
                    emit_hv(prev, first=False, last=True)
                    if DEBUG and p == 0:
                        dsb = bst.tile([P, 2], F32, tag="dsb", name="dsb")
                        nc.vector.tensor_copy(dsb[:, 0:1], pd[0][:])
                        nc.vector.tensor_copy(dsb[:, 1:2], pd[1][:])
                        nc.sync.dma_start(out=dbg_den[:], in_=dsb[:])
                        hsb0 = bst.tile([P, 512], F32, tag="hsb0", name="hsb0")
                        nc.vector.tensor_copy(hsb0[:], ph[(0, 0)][:])
                        nc.sync.dma_start(out=dbg_h00[:], in_=hsb0[:])

                    # evict pair
                    for sl in range(2):
                        i = 2 * p + sl
                        rd = bst.tile([P, 1], F32, tag="rd")
                        nc.vector.reciprocal(rd[:], pd[sl][:])
                        alf = bst.tile([P, H], F32, tag="alf")
                        nc.vector.tensor_copy(alf[:], alpha[i][:])
                        hsb = bst.tile([P, H], F32, tag="hsb")
                        for hc in range(2):
                            nc.scalar.activation(
                                hsb[:, hc * 512:(hc + 1) * 512],
                                ph[(sl, hc)][:], AFT.Copy, scale=rd[:])
                        nc.vector.tensor_sub(hsb[:], hsb[:], xs[sl][:])
                        nc.vector.tensor_mul(hsb[:], hsb[:], alf[:])
                        ho = bst.tile([P, H], BF16, tag="ho")
                        nc.vector.tensor_add(ho[:], hsb[:], xs[sl][:])
                        nc.sync.dma_start(out=out[i * P:(i + 1) * P, :],
                                          in_=ho[:])

    import os
    if os.environ.get("NO_WAIT_SPLIT") != "1":
        split_multi_waits(nc, limit=1, dma_limit=1)
    return nc


_NC_CACHE = None
_LAST_IN_MAPS = None


def _get_nc():
    global _NC_CACHE
    if _NC_CACHE is None:
        _NC_CACHE = build_nc()
    return _NC_CACHE


def _slab(A):
    """[o,h] weight -> per-m stationary slabs: slab[128m+p, 128j+c] = A[128m+c, 128j+p]."""
    B4 = A.reshape(8, P, 8, P)                 # [m, c, j, p]
    return np.ascontiguousarray(
        B4.transpose(0, 3, 2, 1).reshape(H, H)).astype(NPBF16)


def prepare_in_maps(hidden_states, memory_state, q_w, k_w, norm_w, gate_w,
                    gate_b):
    hidden_states = np.asarray(hidden_states, dtype=np.float32)
    memory_state = np.asarray(memory_state, dtype=np.float32)
    q_w = np.asarray(q_w, dtype=np.float32)
    k_w = np.asarray(k_w, dtype=np.float32)
    norm_w = np.asarray(norm_w, dtype=np.float32)
    gate_w = np.asarray(gate_w, dtype=np.float32)
    gate_b = np.asarray(gate_b, dtype=np.float32)

    qw_slab = _slab(q_w * norm_w[None, :])
    kw_slab = _slab(k_w * norm_w[None, :])
    gw = np.ascontiguousarray(gate_w.T).astype(NPBF16)
    b_row = np.ascontiguousarray(gate_b[None, :]).astype(NPBF16)
    w_bc = np.ascontiguousarray(
        np.broadcast_to(norm_w, (P, H))).astype(NPBF16)
    tri = np.where(np.arange(P)[None, :] >= np.arange(P)[:, None],
                   np.float32(0.0), np.float32(NEG)).astype(np.float32)
    tri2 = np.concatenate([tri, np.zeros((P, P), np.float32)], axis=1)

    in_maps = []
    for c in range(N_CORES):
        b, h = divmod(c, 2)
        xb = hidden_states[b]                       # [2048, H]
        blocks = xb.reshape(NKV, P, H)
        lk_order = []
        for i in range(8):
            lk_order += [2 * i + h, 2 * i + (1 - h)]
        x_lk = blocks[lk_order].reshape(2048, H)
        own = blocks[[2 * i + h for i in range(8)]].reshape(1024, H)
        memb = memory_state[b]                      # [64, H]
        memT_slab = np.ascontiguousarray(
            memb.reshape(T_MEM, 8, P).transpose(2, 1, 0).reshape(P, 512)
        ).astype(NPBF16)
        nxt_col = np.full((P, P), NEG if h == 0 else 0.0, np.float32)
        nxt2 = np.concatenate([nxt_col, np.zeros((P, P), np.float32)], axis=1)
        in_maps.append({
            "x_lk": x_lk.astype(NPBF16),
            "xT_lk": np.ascontiguousarray(x_lk.T).astype(NPBF16),
            "xT_own": np.ascontiguousarray(own.T).astype(NPBF16),
            "mem": memb.astype(NPBF16),
            "memT_slab": memT_slab,
            "qw_slab": qw_slab, "kw_slab": kw_slab, "gw": gw,
            "b_row": b_row, "w_bc": w_bc,
            "tri2": tri2, "nxt2": nxt2,
            "onesc": np.ones((P, 1), NPBF16),
            "onesr_b": np.ones((1, P), NPBF16),
            "onesr_f": np.ones((1, P), np.float32),
        })
    return in_maps


def kernel(**inputs):
    in_maps = prepare_in_maps(**inputs)
    global _LAST_IN_MAPS
    _LAST_IN_MAPS = in_maps
    nc = _get_nc()
    res = run_bass_kernel_spmd(nc, in_maps, list(range(N_CORES)))
    out = np.empty((B_FULL, S_FULL, H), dtype=np.float32)
    for c in range(N_CORES):
        b, h = divmod(c, 2)
        r = res.results[c]["out"].astype(np.float32)
        for i in range(8):
            g = 2 * i + h
            out[b, g * P:(g + 1) * P] = r[i * P:(i + 1) * P]
    return out


# revision 24
# speedup vs baseline: 1.7797x; 1.0582x over previous
"""Trainium2 Bass kernel for nn_MemResProjections (memory-residual attention).

Reference computation (B=4, S=2048, K=64, H=1024, fp32):
    normed = rmsnorm(hidden) * norm_w
    v_pool = concat([normed, memory], axis=1)            # (B, S+K, H)
    q = normed @ q_w.T ; k = v_pool @ k_w.T
    logits = q @ k.T / sqrt(H)  with causal mask on the local S block,
    memory columns fully visible
    attn = softmax(logits); h~ = attn @ v_pool
    alpha = sigmoid(hidden @ gate_w.T + gate_b)
    out = (1-alpha)*hidden + alpha*h~

Sharding: 8 cores = (batch b, parity h) pairs.  Core (b,h) owns the 8
query blocks {h, h+2, ..., h+14} (128 rows each) of batch b.  KV blocks are
laid out per-core in a "local" interleaved order  [own_0, oth_0, own_1,
oth_1, ...]  so that own query block i needs exactly the kv-local prefix
0..2i+1 on every core -- one uniform program, per-core data.  The single
per-core mask difference (is the interleaved neighbour before or after me?)
is a data tile (nxt2: -1e30 for h=0, 0 for h=1).

All activations/weights move as bf16 (host-prepped, incl. transposed copies
of x so no PE transposes are needed); psum accumulation fp32; rmsnorm stats
on-device in fp32.  No DRAM spills: v, kT, qT, alpha stay SBUF-resident.
rstd is folded in at PSUM eviction via a partition-broadcast tile built
with a tiny ones-matmul.  Phase order keeps PE hot: stats (ACT) -> kT ->
gate -> qT -> v scaling -> attention; weight/xT loads ride the scalar
HWDGE ring so the sync ring serves the x stripes immediately.
"""
import numpy as np
import ml_dtypes

import concourse.bass as bass
import concourse.mybir as mybir
import concourse.tile as tile
from concourse.bass_utils import run_bass_kernel_spmd

BF16 = mybir.dt.bfloat16
F32 = mybir.dt.float32
F32R = mybir.dt.float32r
AFT = mybir.ActivationFunctionType
NPBF16 = ml_dtypes.bfloat16

P = 128
H = 1024
NJ = H // P           # 8 h-blocks
NKV = 16              # local kv blocks (2048 rows)
T_MEM = 64
SCALE = 1.0 / 32.0    # 1/sqrt(H)
EPS = 1e-6
NEG = -1.0e30

N_CORES = 8
B_FULL, S_FULL = 4, 2048


# ---------------------------------------------------------------- walrus fix
ENGINE_ATTR = {
    mybir.EngineType.PE: "tensor",
    mybir.EngineType.Activation: "scalar",
    mybir.EngineType.DVE: "vector",
    mybir.EngineType.Pool: "gpsimd",
    mybir.EngineType.SP: "sync",
}
DMA_OPS = ("InstDMACopy", "InstDMATranspose", "InstTensorLoad", "InstTensorSave",
           "InstCollectiveCompute")


def split_multi_waits(nc, limit=1, dma_limit=None):
    """This walrus build rejects engine instructions carrying more than one
    sem wait; hoist extras onto same-engine NOPs inserted just before."""
    n_split = 0
    for f in nc.m.functions:
        for blk in f.blocks:
            il = blk.instructions
            i = 0
            while i < len(il):
                ins = il[i]
                is_dma = type(ins).__name__ in DMA_OPS
                lim = dma_limit if is_dma else limit
                si = ins.sync_info
                waits = list(si.on_wait) if si is not None and si.on_wait else []
                if lim is not None and len(waits) > lim:
                    keep, extra = waits[:lim], waits[lim:]
                    si.on_wait.clear()
                    for w in keep:
                        si.on_wait.append(w)
                    eng = getattr(nc, ENGINE_ATTR[ins.engine])
                    for w in extra:
                        nop = eng.nop(nofuse=True, hint="wait_split")
                        nop.wait_op(bass.SemaphoreHandle(w.ant_name, w.id),
                                    w.wait_value, "sem-ge")
                        popped = nc.cur_bb.bb.instructions.pop()
                        assert popped.name == nop.ins.name
                        il.insert(i, nop.ins)
                        i += 1
                        n_split += 1
                i += 1
    return n_split


# ---------------------------------------------------------------- program
def build_nc():
    nc = bass.Bass()
    dp = lambda n, shp, dt: nc.declare_dram_parameter(n, shp, dt, isOutput=False)
    x_lk = dp("x_lk", [2048, H], BF16)        # natural, local-kv row order
    xT_lk = dp("xT_lk", [H, 2048], BF16)      # transposed, same col order
    xT_own = dp("xT_own", [H, 1024], BF16)    # own columns only
    mem = dp("mem", [T_MEM, H], BF16)
    memT_slab = dp("memT_slab", [P, 512], BF16)   # [p, 64j+t] = mem[t, 128j+p]
    qw_slab = dp("qw_slab", [H, H], BF16)     # slab m rows: [p, 128j+c] = qwT_w[128j+p, 128m+c]
    kw_slab = dp("kw_slab", [H, H], BF16)
    gw = dp("gw", [H, H], BF16)               # gate_w.T  [h, o]
    b_row = dp("b_row", [1, H], BF16)         # gate_b
    w_bc = dp("w_bc", [P, H], BF16)           # norm_w broadcast
    tri2 = dp("tri2", [P, 256], F32)          # [tri | 0]
    nxt2 = dp("nxt2", [P, 256], F32)          # [nextb | 0], nextb = -1e30 (h=0) / 0 (h=1)
    onesc = dp("onesc", [P, 1], BF16)
    onesr_b = dp("onesr_b", [1, P], BF16)
    onesr_f = dp("onesr_f", [1, P], F32)
    out = nc.declare_dram_parameter("out", [1024, H], BF16, isOutput=True)
    import os as _os
    DEBUG = _os.environ.get("DEBUG_KERNEL") == "1"
    if DEBUG:
        dout = lambda n, shp: nc.declare_dram_parameter(n, shp, F32,
                                                        isOutput=True)
        dbg_rstd = dout("dbg_rstd", [P, NKV])
        dbg_bc = dout("dbg_bc", [P, 2048])
        dbg_al0 = dout("dbg_al0", [P, H])
        dbg_kt0 = nc.declare_dram_parameter("dbg_kt0", [P, 2048 + T_MEM],
                                            BF16, isOutput=True)
        dbg_qt0 = nc.declare_dram_parameter("dbg_qt0", [P, 1024], BF16,
                                            isOutput=True)
        dbg_v0 = nc.declare_dram_parameter("dbg_v0", [P, H], BF16,
                                           isOutput=True)
        dbg_et0 = nc.declare_dram_parameter("dbg_et0", [P, 256], BF16,
                                            isOutput=True)
        dbg_etm = nc.declare_dram_parameter("dbg_etm", [P, 256], BF16,
                                            isOutput=True)
        dbg_den = dout("dbg_den", [P, 2])
        dbg_h00 = dout("dbg_h00", [P, 512])

    with tile.TileContext(nc) as tc:
        from contextlib import ExitStack
        with ExitStack() as ctx:
            # ---- long-lived pools (v/qT open after phase A to keep the
            # peak under the SBUF cap -- pools reserve for their whole scope)
            const = ctx.enter_context(tc.tile_pool(name="const", bufs=1))
            ktp = ctx.enter_context(tc.tile_pool(name="ktp", bufs=1))
            apool = ctx.enter_context(tc.tile_pool(name="apool", bufs=1))

            eps_t = const.tile([P, 1], F32)
            nc.vector.memset(eps_t[:], EPS)
            rstd_nat = const.tile([P, NKV], F32)
            sdt_all = const.tile([P, NKV], F32)
            scl_nat = const.tile([P, NKV], F32)
            rrow_own = const.tile([1, 1024], F32R)
            std_row = const.tile([1, 1024], F32)
            rrow_f = const.tile([1, 1024], F32)
            rstd_bc_own = const.tile([P, 1024], F32)

            kT = [ktp.tile([P, 2048 + T_MEM], BF16, tag=f"kT{m}", name=f"kT{m}")
                  for m in range(NJ)]
            alpha = [apool.tile([P, H], F32, tag=f"al{i}", name=f"al{i}")
                     for i in range(8)]

            # xT_lk tiles live through kT + gate.  Only the 8 xT loads ride
            # the scalar HWDGE ring (so ACT is free after ~5us); everything
            # else is issued on the sync ring in criticality order.
            xop = ctx.enter_context(tc.tile_pool(name="xop", bufs=1))
            with tc.tile_pool(name="xtpool", bufs=1) as xtpool, \
                 tc.tile_pool(name="sqo", bufs=1) as sqo, \
                 tc.tile_pool(name="kwp", bufs=1) as kwp, \
                 tc.tile_pool(name="gwp", bufs=1) as gwp:
                xT_t = [xtpool.tile([P, 2048], BF16, tag=f"xT{j}", name=f"xT{j}")
                        for j in range(NJ)]
                for j in range(NJ):
                    nc.sync.dma_start(out=xT_t[j][:],
                                      in_=xT_lk[j * P:(j + 1) * P, :])
                kw_t = [kwp.tile([P, H], BF16, tag=f"kw{m}", name=f"kw{m}")
                        for m in range(NJ)]
                for m in range(NJ):
                    nc.scalar.dma_start(out=kw_t[m][:],
                                        in_=kw_slab[m * P:(m + 1) * P, :])
                memT_t = const.tile([P, 512], BF16)
                nc.scalar.dma_start(out=memT_t[:], in_=memT_slab[:])
                onesc_t = const.tile([P, 1], BF16)
                nc.scalar.dma_start(out=onesc_t[:], in_=onesc[:])
                onesrf_t = const.tile([1, P], F32R)
                nc.scalar.dma_start(out=onesrf_t[:], in_=onesr_f[:].bitcast(F32R))

                # ---- A1: rmsnorm stats on ACT only (reciprocals deferred so
                # the DVE stream is free for kT evictions)
                with tc.tile_pool(name="a1s", bufs=3) as a1s:
                    for l in range(NKV):
                        xt = a1s.tile([P, H], BF16, tag="xt")
                        nc.sync.dma_start(out=xt[:],
                                          in_=x_lk[l * P:(l + 1) * P, :])
                        sq = a1s.tile([P, H], BF16, tag="sq", bufs=2)
                        ss = a1s.tile([P, 1], F32, tag="ss")
                        nc.scalar.activation(sq[:], xt[:], AFT.Square,
                                             accum_out=ss[:])
                        nc.scalar.activation(sdt_all[:, l:l + 1], ss[:],
                                             AFT.Sqrt, scale=1.0 / H,
                                             bias=eps_t[:])
                xo_t = [xop.tile([P, 1024], BF16, tag=f"xo{j}", name=f"xo{j}")
                        for j in range(NJ)]
                for j in range(NJ):
                    nc.sync.dma_start(out=xo_t[j][:],
                                      in_=xT_own[j * P:(j + 1) * P, :])
                gw_t = [gwp.tile([P, H], BF16, tag=f"gw{j}", name=f"gw{j}")
                        for j in range(NJ)]
                for j in range(NJ):
                    nc.sync.dma_start(out=gw_t[j][:],
                                      in_=gw[j * P:(j + 1) * P, :])
                onesrb_t = const.tile([1, P], BF16)
                nc.sync.dma_start(out=onesrb_t[:], in_=onesr_b[:])
                b_row_t = const.tile([1, H], BF16)
                nc.sync.dma_start(out=b_row_t[:], in_=b_row[:])
                # squares of own transposed cols for the row-form stats
                sq_own = [sqo.tile([P, 1024], BF16, tag=f"sqo{j}", name=f"sqo{j}")
                          for j in range(NJ)]
                for j in range(NJ):
                    nc.scalar.activation(sq_own[j][:], xo_t[j][:],
                                         AFT.Square)

                # ---- kT projection (row-form q-rstd stats interleaved so
                # rstd_bc_own is ready long before the qT evictions)
                with tc.tile_pool(name="rowp", bufs=1, space="PSUM") as rowp, \
                     tc.tile_pool(name="pps", bufs=4, space="PSUM") as pps:
                    for m in range(NJ):
                        kwm = kw_t[m]
                        if m == 3:
                            for c in range(2):
                                pv = rowp.tile([1, 512], F32, tag="pv")
                                for j in range(NJ):
                                    nc.tensor.matmul(
                                        pv[:], onesc_t[:],
                                        sq_own[j][:, c * 512:(c + 1) * 512],
                                        start=(j == 0), stop=(j == NJ - 1))
                                nc.scalar.activation(
                                    std_row[0:1, c * 512:(c + 1) * 512],
                                    pv[:], AFT.Sqrt, scale=1.0 / H,
                                    bias=eps_t[0:1, :])
                            nc.vector.reciprocal(rrow_f[:], std_row[:])
                            nc.scalar.activation(rrow_own[:], rrow_f[:],
                                                 AFT.Copy)
                        if m == 4:
                            for c in range(2):
                                pb = rowp.tile([P, 512], F32, tag="pbc")
                                nc.tensor.matmul(
                                    pb[:], onesrf_t[:],
                                    rrow_own[0:1, c * 512:(c + 1) * 512],
                                    start=True, stop=True)
                                nc.vector.tensor_copy(
                                    rstd_bc_own[:, c * 512:(c + 1) * 512],
                                    pb[:])
                        for c in range(4):
                            pk = pps.tile([P, 512], F32, tag="pk")
                            for j in range(NJ):
                                nc.tensor.matmul(
                                    pk[:], kwm[:, j * P:(j + 1) * P],
                                    xT_t[j][:, c * 512:(c + 1) * 512],
                                    start=(j == 0), stop=(j == NJ - 1))
                            nc.vector.tensor_copy(
                                kT[m][:, c * 512:(c + 1) * 512], pk[:])
                        pkm = pps.tile([P, T_MEM], F32, tag="pkm", bufs=2)
                        for j in range(NJ):
                            nc.tensor.matmul(
                                pkm[:], kwm[:, j * P:(j + 1) * P],
                                memT_t[:, j * T_MEM:(j + 1) * T_MEM],
                                start=(j == 0), stop=(j == NJ - 1))
                        nc.vector.tensor_copy(kT[m][:, 2048:2048 + T_MEM],
                                              pkm[:])

                # ---- gate (uses xT_lk as lhsT; alpha stored bf16)
                with tc.tile_pool(name="gps", bufs=2, space="PSUM") as gps:
                    for i in range(8):
                        pg = [gps.tile([P, 512], F32, tag=f"pg{oc}",
                                       name=f"pg{i}_{oc}")
                              for oc in range(2)]
                        for j in range(NJ):
                            for oc in range(2):
                                nc.tensor.matmul(
                                    pg[oc][:],
                                    xT_t[j][:, 2 * i * P:(2 * i + 1) * P],
                                    gw_t[j][:, oc * 512:(oc + 1) * 512],
                                    start=(j == 0), stop=False)
                        for oc in range(2):
                            nc.tensor.matmul(
                                pg[oc][:], onesrb_t[:],
                                b_row_t[0:1, oc * 512:(oc + 1) * 512],
                                start=False, stop=True)
                            nc.scalar.activation(
                                alpha[i][:, oc * 512:(oc + 1) * 512],
                                pg[oc][:], AFT.Sigmoid)

            vpool = ctx.enter_context(tc.tile_pool(name="vpool", bufs=1))
            qtp = ctx.enter_context(tc.tile_pool(name="qtp", bufs=1))
            v = [vpool.tile([P, H], BF16, tag=f"v{l}", name=f"v{l}")
                 for l in range(NKV)]
            qT = [qtp.tile([P, 1024], BF16, tag=f"qT{m}", name=f"qT{m}")
                  for m in range(NJ)]

            # ---- qT projection (xT_lk released; own transposed cols)
            with tc.tile_pool(name="wsl2", bufs=2) as wsl2, \
                 tc.tile_pool(name="pps2", bufs=4, space="PSUM") as pps2:
                for l in range(NKV):
                    nc.vector.reciprocal(rstd_nat[:, l:l + 1],
                                         sdt_all[:, l:l + 1])
                nc.vector.tensor_scalar_mul(scl_nat[:], rstd_nat[:], SCALE)
                for m in range(NJ):
                    qwm = wsl2.tile([P, H], BF16, tag="qwm")
                    nc.scalar.dma_start(out=qwm[:],
                                        in_=qw_slab[m * P:(m + 1) * P, :])
                    for c in range(2):
                        pq = pps2.tile([P, 512], F32, tag="pq")
                        for j in range(NJ):
                            nc.tensor.matmul(
                                pq[:], qwm[:, j * P:(j + 1) * P],
                                xo_t[j][:, c * 512:(c + 1) * 512],
                                start=(j == 0), stop=(j == NJ - 1))
                        nc.vector.tensor_mul(
                            qT[m][:, c * 512:(c + 1) * 512], pq[:],
                            rstd_bc_own[:, c * 512:(c + 1) * 512])

            # ---- v pass: v = x * rstd * w  (x re-streamed)
            with tc.tile_pool(name="vps", bufs=3) as vps:
                w_bc_t = const.tile([P, H], BF16)
                nc.sync.dma_start(out=w_bc_t[:], in_=w_bc[:])
                v_mem = const.tile([T_MEM, H], BF16)
                nc.sync.dma_start(out=v_mem[:], in_=mem[:])
                tri2_t = const.tile([P, 256], F32)
                nc.sync.dma_start(out=tri2_t[:], in_=tri2[:])
                nxt2_t = const.tile([P, 256], F32)
                nc.sync.dma_start(out=nxt2_t[:], in_=nxt2[:])
                for l in range(NKV):
                    x2 = vps.tile([P, H], BF16, tag="x2")
                    nc.sync.dma_start(out=x2[:],
                                      in_=x_lk[l * P:(l + 1) * P, :])
                    nc.scalar.activation(v[l][:], x2[:], AFT.Copy,
                                         scale=rstd_nat[:, l:l + 1])
                    nc.vector.tensor_mul(v[l][:], v[l][:], w_bc_t[:])

            if DEBUG:
                nc.sync.dma_start(out=dbg_rstd[:], in_=rstd_nat[:])
                nc.sync.dma_start(
                    out=dbg_bc[:, 0:1024], in_=rstd_bc_own[:])
                nc.sync.dma_start(
                    out=dbg_bc[:, 1024:1040], in_=rstd_nat[:])
                nc.sync.dma_start(out=dbg_al0[:], in_=alpha[0][:])
                nc.sync.dma_start(out=dbg_kt0[:], in_=kT[0][:])
                nc.sync.dma_start(out=dbg_qt0[:], in_=qT[0][:])
                nc.sync.dma_start(out=dbg_v0[:], in_=v[0][:])

            # ================= phase B: attention (4 balanced pairs)
            with tc.tile_pool(name="bst", bufs=2) as bst, \
                 tc.tile_pool(name="bet", bufs=6) as bet, \
                 tc.tile_pool(name="bps", bufs=2, space="PSUM") as bps, \
                 tc.tile_pool(name="bph", bufs=1, space="PSUM") as bph, \
                 tc.tile_pool(name="bpd", bufs=1, space="PSUM") as bpd:
                for p in range(4):
                    q0 = 256 * p
                    xs = []
                    for sl in range(2):
                        xsb = bst.tile([P, H], BF16, tag=f"xsb{sl}")
                        i = 2 * p + sl
                        nc.sync.dma_start(
                            out=xsb[:],
                            in_=x_lk[2 * i * P:(2 * i + 1) * P, :])
                        xst = bst.tile([P, H], F32, tag=f"xs{sl}")
                        nc.vector.tensor_copy(xst[:], xsb[:])
                        xs.append(xst)
                    ph = {(sl, hc): bph.tile([P, 512], F32, tag=f"ph{sl}{hc}",
                                             name=f"ph{sl}{hc}")
                          for sl in range(2) for hc in range(2)}
                    # NB: separate tiles => separate PSUM banks.  start=True
                    # zeroes a whole 2KB zero region, so the two q-blocks'
                    # denominators must not share a bank.
                    pd = [bpd.tile([P, 1], F32, tag=f"pd{sl}", name=f"pd{sl}")
                          for sl in range(2)]

                    # tasks: ("sh", t) both q-blocks; ("so", t) second only; mem
                    tasks = ([("sh", t) for t in range(4 * p + 2)]
                             + [("so", 4 * p + 2), ("so", 4 * p + 3)]
                             + [("mem", 0)])

                    def emit_scores(task):
                        kind, t = task
                        if kind == "sh":
                            ps = bps.tile([P, 256], F32, tag="ps")
                            for m in range(NJ):
                                nc.tensor.matmul(
                                    ps[:], kT[m][:, t * P:(t + 1) * P],
                                    qT[m][:, q0:q0 + 256],
                                    start=(m == 0), stop=(m == NJ - 1))
                            if t == 4 * p:
                                nc.vector.tensor_add(ps[:], ps[:], tri2_t[:])
                            elif t == 4 * p + 1:
                                nc.vector.tensor_add(ps[:], ps[:], nxt2_t[:])
                            et = bet.tile([P, 256], BF16, tag="et")
                            nc.scalar.activation(et[:], ps[:], AFT.Exp,
                                                 scale=scl_nat[:, t:t + 1])
                            return (kind, t, et, P)
                        if kind == "so":
                            ps = bps.tile([P, 256], F32, tag="ps")
                            for m in range(NJ):
                                nc.tensor.matmul(
                                    ps[:, 0:P], kT[m][:, t * P:(t + 1) * P],
                                    qT[m][:, q0 + P:q0 + 256],
                                    start=(m == 0), stop=(m == NJ - 1))
                            bias = tri2_t if t == 4 * p + 2 else nxt2_t
                            nc.vector.tensor_add(ps[:, 0:P], ps[:, 0:P],
                                                 bias[:, 0:P])
                            et = bet.tile([P, 256], BF16, tag="et")
                            nc.scalar.activation(et[:, 0:P], ps[:, 0:P],
                                                 AFT.Exp,
                                                 scale=scl_nat[:, t:t + 1])
                            return (kind, t, et, P)
                        # mem
                        ps = bps.tile([P, 256], F32, tag="ps")
                        for m in range(NJ):
                            nc.tensor.matmul(
                                ps[:T_MEM, :], kT[m][:, 2048:2048 + T_MEM],
                                qT[m][:, q0:q0 + 256],
                                start=(m == 0), stop=(m == NJ - 1))
                        et = bet.tile([P, 256], BF16, tag="et")
                        nc.scalar.activation(et[:T_MEM, :], ps[:T_MEM, :],
                                             AFT.Exp, scale=SCALE)
                        return (kind, t, et, T_MEM)

                    def emit_hv(sc, first, last):
                        kind, t, et, rows = sc
                        if kind == "sh":
                            vt, sls = v[t], (0, 1)
                        elif kind == "so":
                            vt, sls = v[t], (1,)
                        else:
                            vt, sls = v_mem, (0, 1)
                        for sl in sls:
                            if kind == "so":
                                lhs = et[:rows, 0:P]
                            else:
                                lhs = et[:rows, sl * P:(sl + 1) * P]
                            for hc in range(2):
                                nc.tensor.matmul(
                                    ph[(sl, hc)][:], lhs,
                                    vt[:rows, hc * 512:(hc + 1) * 512],
                                    start=first, stop=last,
                                    skip_group_check=True)
                            nc.tensor.matmul(
                                pd[sl][:], lhs, onesc_t[:rows, :],
                                start=first, stop=last,
                                skip_group_check=True)

                    scs = []
                    for ti, task in enumerate(tasks):
                        sc = emit_scores(task)
                        if DEBUG and p == 0 and ti == 0:
                            nc.sync.dma_start(out=dbg_et0[:], in_=sc[2][:])
                        if DEBUG and p == 0 and task[0] == "mem":
                            nc.sync.dma_start(out=dbg_etm[:], in_=sc[2][:])
                        scs.append(sc)
                        if len(scs) > 2:
                            old_sc = scs.pop(0)
                            emit_hv(old_sc,
                                    first=(old_sc[1] == 0 and old_sc[0] == "sh"),
                                    last=False)
                    while scs:
                        old_sc = scs.pop(0)
                        emit_hv(old_sc,
                                first=(old_sc[1] == 0 and old_sc[0] == "sh"),
                                last=(not scs))
                    if DEBUG and p == 0:
                        dsb = bst.tile([P, 2], F32, tag="dsb", name="dsb")
                        nc.vector.tensor_copy(dsb[:, 0:1], pd[0][:])
                        nc.vector.tensor_copy(dsb[:, 1:2], pd[1][:])
                        nc.sync.dma_start(out=dbg_den[:], in_=dsb[:])
                        hsb0 = bst.tile([P, 512], F32, tag="hsb0", name="hsb0")
                        nc.vector.tensor_copy(hsb0[:], ph[(0, 0)][:])
                        nc.sync.dma_start(out=dbg_h00[:], in_=hsb0[:])

                    # evict pair
                    for sl in range(2):
                        i = 2 * p + sl
                        rd = bst.tile([P, 1], F32, tag="rd")
                        nc.vector.reciprocal(rd[:], pd[sl][:])
                        hsb = bst.tile([P, H], F32, tag="hsb")
                        for hc in range(2):
                            nc.vector.tensor_scalar_mul(
                                hsb[:, hc * 512:(hc + 1) * 512],
                                ph[(sl, hc)][:], rd[:])
                        nc.vector.tensor_sub(hsb[:], hsb[:], xs[sl][:])
                        nc.vector.tensor_mul(hsb[:], hsb[:], alpha[i][:])
                        ho = bst.tile([P, H], BF16, tag="ho")
                        nc.vector.tensor_add(ho[:], hsb[:], xs[sl][:])
                        nc.sync.dma_start(out=out[i * P:(i + 1) * P, :],
                                          in_=ho[:])

    import os
    if os.environ.get("NO_WAIT_SPLIT") != "1":
        split_multi_waits(nc, limit=1, dma_limit=1)
    return nc


_NC_CACHE = None
_LAST_IN_MAPS = None


def _get_nc():
    global _NC_CACHE
    if _NC_CACHE is None:
        _NC_CACHE = build_nc()
    return _NC_CACHE


def _slab(A):
    """[o,h] weight -> per-m stationary slabs: slab[128m+p, 128j+c] = A[128m+c, 128j+p]."""
    B4 = A.reshape(8, P, 8, P)                 # [m, c, j, p]
    return np.ascontiguousarray(
        B4.transpose(0, 3, 2, 1).reshape(H, H)).astype(NPBF16)


def prepare_in_maps(hidden_states, memory_state, q_w, k_w, norm_w, gate_w,
                    gate_b):
    hidden_states = np.asarray(hidden_states, dtype=np.float32)
    memory_state = np.asarray(memory_state, dtype=np.float32)
    q_w = np.asarray(q_w, dtype=np.float32)
    k_w = np.asarray(k_w, dtype=np.float32)
    norm_w = np.asarray(norm_w, dtype=np.float32)
    gate_w = np.asarray(gate_w, dtype=np.float32)
    gate_b = np.asarray(gate_b, dtype=np.float32)

    qw_slab = _slab(q_w * norm_w[None, :])
    kw_slab = _slab(k_w * norm_w[None, :])
    gw = np.ascontiguousarray(gate_w.T).astype(NPBF16)
    b_row = np.ascontiguousarray(gate_b[None, :]).astype(NPBF16)
    w_bc = np.ascontiguousarray(
        np.broadcast_to(norm_w, (P, H))).astype(NPBF16)
    tri = np.where(np.arange(P)[None, :] >= np.arange(P)[:, None],
                   np.float32(0.0), np.float32(NEG)).astype(np.float32)
    tri2 = np.concatenate([tri, np.zeros((P, P), np.float32)], axis=1)

    in_maps = []
    for c in range(N_CORES):
        b, h = divmod(c, 2)
        xb = hidden_states[b]                       # [2048, H]
        blocks = xb.reshape(NKV, P, H)
        lk_order = []
        for i in range(8):
            lk_order += [2 * i + h, 2 * i + (1 - h)]
        x_lk = blocks[lk_order].reshape(2048, H)
        own = blocks[[2 * i + h for i in range(8)]].reshape(1024, H)
        memb = memory_state[b]                      # [64, H]
        memT_slab = np.ascontiguousarray(
            memb.reshape(T_MEM, 8, P).transpose(2, 1, 0).reshape(P, 512)
        ).astype(NPBF16)
        nxt_col = np.full((P, P), NEG if h == 0 else 0.0, np.float32)
        nxt2 = np.concatenate([nxt_col, np.zeros((P, P), np.float32)], axis=1)
        in_maps.append({
            "x_lk": x_lk.astype(NPBF16),
            "xT_lk": np.ascontiguousarray(x_lk.T).astype(NPBF16),
            "xT_own": np.ascontiguousarray(own.T).astype(NPBF16),
            "mem": memb.astype(NPBF16),
            "memT_slab": memT_slab,
            "qw_slab": qw_slab, "kw_slab": kw_slab, "gw": gw,
            "b_row": b_row, "w_bc": w_bc,
            "tri2": tri2, "nxt2": nxt2,
            "onesc": np.ones((P, 1), NPBF16),
            "onesr_b": np.ones((1, P), NPBF16),
            "onesr_f": np.ones((1, P), np.float32),
        })
    return in_maps


def kernel(**inputs):
    in_maps = prepare_in_maps(**inputs)
    global _LAST_IN_MAPS
    _LAST_IN_MAPS = in_maps
    nc = _get_nc()
    res = run_bass_kernel_spmd(nc, in_maps, list(range(N_CORES)))
    out = np.empty((B_FULL, S_FULL, H), dtype=np.float32)
    for c in range(N_CORES):
        b, h = divmod(c, 2)
        r = res.results[c]["out"].astype(np.float32)
        for i in range(8):
            g = 2 * i + h
            out[b, g * P:(g + 1) * P] = r[i * P:(i + 1) * P]
    return out


# revision 25
# speedup vs baseline: 1.7918x; 1.0068x over previous
"""Trainium2 Bass kernel for nn_MemResProjections (memory-residual attention).

Reference computation (B=4, S=2048, K=64, H=1024, fp32):
    normed = rmsnorm(hidden) * norm_w
    v_pool = concat([normed, memory], axis=1)            # (B, S+K, H)
    q = normed @ q_w.T ; k = v_pool @ k_w.T
    logits = q @ k.T / sqrt(H)  with causal mask on the local S block,
    memory columns fully visible
    attn = softmax(logits); h~ = attn @ v_pool
    alpha = sigmoid(hidden @ gate_w.T + gate_b)
    out = (1-alpha)*hidden + alpha*h~

Sharding: 8 cores = (batch b, parity h) pairs.  Core (b,h) owns the 8
query blocks {h, h+2, ..., h+14} (128 rows each) of batch b.  KV blocks are
laid out per-core in a "local" interleaved order  [own_0, oth_0, own_1,
oth_1, ...]  so that own query block i needs exactly the kv-local prefix
0..2i+1 on every core -- one uniform program, per-core data.  The single
per-core mask difference (is the interleaved neighbour before or after me?)
is a data tile (nxt2: -1e30 for h=0, 0 for h=1).

All activations/weights move as bf16 (host-prepped, incl. transposed copies
of x so no PE transposes are needed); psum accumulation fp32; rmsnorm stats
on-device in fp32.  No DRAM spills: v, kT, qT, alpha stay SBUF-resident.
rstd is folded in at PSUM eviction via a partition-broadcast tile built
with a tiny ones-matmul.  Phase order keeps PE hot: stats (ACT) -> kT ->
gate -> qT -> v scaling -> attention; weight/xT loads ride the scalar
HWDGE ring so the sync ring serves the x stripes immediately.
"""
import numpy as np
import ml_dtypes

import concourse.bass as bass
import concourse.mybir as mybir
import concourse.tile as tile
from concourse.bass_utils import run_bass_kernel_spmd

BF16 = mybir.dt.bfloat16
F32 = mybir.dt.float32
F32R = mybir.dt.float32r
AFT = mybir.ActivationFunctionType
NPBF16 = ml_dtypes.bfloat16

P = 128
H = 1024
NJ = H // P           # 8 h-blocks
NKV = 16              # local kv blocks (2048 rows)
T_MEM = 64
SCALE = 1.0 / 32.0    # 1/sqrt(H)
EPS = 1e-6
NEG = -1.0e30

N_CORES = 8
B_FULL, S_FULL = 4, 2048


# ---------------------------------------------------------------- walrus fix
ENGINE_ATTR = {
    mybir.EngineType.PE: "tensor",
    mybir.EngineType.Activation: "scalar",
    mybir.EngineType.DVE: "vector",
    mybir.EngineType.Pool: "gpsimd",
    mybir.EngineType.SP: "sync",
}
DMA_OPS = ("InstDMACopy", "InstDMATranspose", "InstTensorLoad", "InstTensorSave",
           "InstCollectiveCompute")


def split_multi_waits(nc, limit=1, dma_limit=None):
    """This walrus build rejects engine instructions carrying more than one
    sem wait; hoist extras onto same-engine NOPs inserted just before."""
    n_split = 0
    for f in nc.m.functions:
        for blk in f.blocks:
            il = blk.instructions
            i = 0
            while i < len(il):
                ins = il[i]
                is_dma = type(ins).__name__ in DMA_OPS
                lim = dma_limit if is_dma else limit
                si = ins.sync_info
                waits = list(si.on_wait) if si is not None and si.on_wait else []
                if lim is not None and len(waits) > lim:
                    keep, extra = waits[:lim], waits[lim:]
                    si.on_wait.clear()
                    for w in keep:
                        si.on_wait.append(w)
                    eng = getattr(nc, ENGINE_ATTR[ins.engine])
                    for w in extra:
                        nop = eng.nop(nofuse=True, hint="wait_split")
                        nop.wait_op(bass.SemaphoreHandle(w.ant_name, w.id),
                                    w.wait_value, "sem-ge")
                        popped = nc.cur_bb.bb.instructions.pop()
                        assert popped.name == nop.ins.name
                        il.insert(i, nop.ins)
                        i += 1
                        n_split += 1
                i += 1
    return n_split


# ---------------------------------------------------------------- program
def build_nc():
    nc = bass.Bass()
    dp = lambda n, shp, dt: nc.declare_dram_parameter(n, shp, dt, isOutput=False)
    x_lk = dp("x_lk", [2048, H], BF16)        # natural, local-kv row order
    xT_lk = dp("xT_lk", [H, 2048], BF16)      # transposed, same col order
    xT_own = dp("xT_own", [H, 1024], BF16)    # own columns only
    mem = dp("mem", [T_MEM, H], BF16)
    memT_slab = dp("memT_slab", [P, 512], BF16)   # [p, 64j+t] = mem[t, 128j+p]
    qw_slab = dp("qw_slab", [H, H], BF16)     # slab m rows: [p, 128j+c] = qwT_w[128j+p, 128m+c]
    kw_slab = dp("kw_slab", [H, H], BF16)
    gw = dp("gw", [H, H], BF16)               # gate_w.T  [h, o]
    b_row = dp("b_row", [1, H], BF16)         # gate_b
    w_bc = dp("w_bc", [P, H], BF16)           # norm_w broadcast
    tri2 = dp("tri2", [P, 256], F32)          # [tri | 0]
    nxt2 = dp("nxt2", [P, 256], F32)          # [nextb | 0], nextb = -1e30 (h=0) / 0 (h=1)
    onesc = dp("onesc", [P, 1], BF16)
    onesr_b = dp("onesr_b", [1, P], BF16)
    onesr_f = dp("onesr_f", [1, P], F32)
    out = nc.declare_dram_parameter("out", [1024, H], BF16, isOutput=True)
    import os as _os
    DEBUG = _os.environ.get("DEBUG_KERNEL") == "1"
    if DEBUG:
        dout = lambda n, shp: nc.declare_dram_parameter(n, shp, F32,
                                                        isOutput=True)
        dbg_rstd = dout("dbg_rstd", [P, NKV])
        dbg_bc = dout("dbg_bc", [P, 2048])
        dbg_al0 = dout("dbg_al0", [P, H])
        dbg_kt0 = nc.declare_dram_parameter("dbg_kt0", [P, 2048 + T_MEM],
                                            BF16, isOutput=True)
        dbg_qt0 = nc.declare_dram_parameter("dbg_qt0", [P, 1024], BF16,
                                            isOutput=True)
        dbg_v0 = nc.declare_dram_parameter("dbg_v0", [P, H], BF16,
                                           isOutput=True)
        dbg_et0 = nc.declare_dram_parameter("dbg_et0", [P, 256], BF16,
                                            isOutput=True)
        dbg_etm = nc.declare_dram_parameter("dbg_etm", [P, 256], BF16,
                                            isOutput=True)
        dbg_den = dout("dbg_den", [P, 2])
        dbg_h00 = dout("dbg_h00", [P, 512])

    with tile.TileContext(nc) as tc:
        from contextlib import ExitStack
        with ExitStack() as ctx:
            # ---- long-lived pools (v/qT open after phase A to keep the
            # peak under the SBUF cap -- pools reserve for their whole scope)
            const = ctx.enter_context(tc.tile_pool(name="const", bufs=1))
            ktp = ctx.enter_context(tc.tile_pool(name="ktp", bufs=1))
            apool = ctx.enter_context(tc.tile_pool(name="apool", bufs=1))

            eps_t = const.tile([P, 1], F32)
            nc.vector.memset(eps_t[:], EPS)
            rstd_nat = const.tile([P, NKV], F32)
            sdt_all = const.tile([P, NKV], F32)
            scl_nat = const.tile([P, NKV], F32)
            rrow_own = const.tile([1, 1024], F32R)
            std_row = const.tile([1, 1024], F32)
            rrow_f = const.tile([1, 1024], F32)
            rstd_bc_own = const.tile([P, 1024], F32)

            kT = [ktp.tile([P, 2048 + T_MEM], BF16, tag=f"kT{m}", name=f"kT{m}")
                  for m in range(NJ)]
            alpha = [apool.tile([P, H], F32, tag=f"al{i}", name=f"al{i}")
                     for i in range(8)]

            # xT_lk tiles live through kT + gate.  Only the 8 xT loads ride
            # the scalar HWDGE ring (so ACT is free after ~5us); everything
            # else is issued on the sync ring in criticality order.
            xop = ctx.enter_context(tc.tile_pool(name="xop", bufs=1))
            with tc.tile_pool(name="xtpool", bufs=1) as xtpool, \
                 tc.tile_pool(name="sqo", bufs=1) as sqo, \
                 tc.tile_pool(name="kwp", bufs=1) as kwp, \
                 tc.tile_pool(name="gwp", bufs=1) as gwp:
                xT_t = [xtpool.tile([P, 2048], BF16, tag=f"xT{j}", name=f"xT{j}")
                        for j in range(NJ)]
                for j in range(NJ):
                    nc.sync.dma_start(out=xT_t[j][:],
                                      in_=xT_lk[j * P:(j + 1) * P, :])
                kw_t = [kwp.tile([P, H], BF16, tag=f"kw{m}", name=f"kw{m}")
                        for m in range(NJ)]
                for m in range(NJ):
                    nc.scalar.dma_start(out=kw_t[m][:],
                                        in_=kw_slab[m * P:(m + 1) * P, :])
                memT_t = const.tile([P, 512], BF16)
                nc.scalar.dma_start(out=memT_t[:], in_=memT_slab[:])
                onesc_t = const.tile([P, 1], BF16)
                nc.scalar.dma_start(out=onesc_t[:], in_=onesc[:])
                onesrf_t = const.tile([1, P], F32R)
                nc.scalar.dma_start(out=onesrf_t[:], in_=onesr_f[:].bitcast(F32R))

                # ---- A1: rmsnorm stats on ACT only (reciprocals deferred so
                # the DVE stream is free for kT evictions)
                with tc.tile_pool(name="a1s", bufs=3) as a1s:
                    for l in range(NKV):
                        xt = a1s.tile([P, H], BF16, tag="xt")
                        nc.sync.dma_start(out=xt[:],
                                          in_=x_lk[l * P:(l + 1) * P, :])
                        sq = a1s.tile([P, H], BF16, tag="sq", bufs=2)
                        ss = a1s.tile([P, 1], F32, tag="ss")
                        nc.scalar.activation(sq[:], xt[:], AFT.Square,
                                             accum_out=ss[:])
                        nc.scalar.activation(sdt_all[:, l:l + 1], ss[:],
                                             AFT.Sqrt, scale=1.0 / H,
                                             bias=eps_t[:])
                xo_t = [xop.tile([P, 1024], BF16, tag=f"xo{j}", name=f"xo{j}")
                        for j in range(NJ)]
                for j in range(NJ):
                    nc.sync.dma_start(out=xo_t[j][:],
                                      in_=xT_own[j * P:(j + 1) * P, :])
                gw_t = [gwp.tile([P, H], BF16, tag=f"gw{j}", name=f"gw{j}")
                        for j in range(NJ)]
                for j in range(NJ):
                    nc.sync.dma_start(out=gw_t[j][:],
                                      in_=gw[j * P:(j + 1) * P, :])
                onesrb_t = const.tile([1, P], BF16)
                nc.sync.dma_start(out=onesrb_t[:], in_=onesr_b[:])
                b_row_t = const.tile([1, H], BF16)
                nc.sync.dma_start(out=b_row_t[:], in_=b_row[:])
                # squares of own transposed cols for the row-form stats
                sq_own = [sqo.tile([P, 1024], BF16, tag=f"sqo{j}", name=f"sqo{j}")
                          for j in range(NJ)]
                for j in range(NJ):
                    nc.scalar.activation(sq_own[j][:], xo_t[j][:],
                                         AFT.Square)

                # ---- kT projection (row-form q-rstd stats interleaved so
                # rstd_bc_own is ready long before the qT evictions)
                with tc.tile_pool(name="rowp", bufs=1, space="PSUM") as rowp, \
                     tc.tile_pool(name="pps", bufs=4, space="PSUM") as pps:
                    for m in range(NJ):
                        kwm = kw_t[m]
                        if m == 3:
                            for c in range(2):
                                pv = rowp.tile([1, 512], F32, tag="pv")
                                for j in range(NJ):
                                    nc.tensor.matmul(
                                        pv[:], onesc_t[:],
                                        sq_own[j][:, c * 512:(c + 1) * 512],
                                        start=(j == 0), stop=(j == NJ - 1))
                                nc.scalar.activation(
                                    std_row[0:1, c * 512:(c + 1) * 512],
                                    pv[:], AFT.Sqrt, scale=1.0 / H,
                                    bias=eps_t[0:1, :])
                            nc.vector.reciprocal(rrow_f[:], std_row[:])
                            nc.scalar.activation(rrow_own[:], rrow_f[:],
                                                 AFT.Copy)
                        if m == 4:
                            for c in range(2):
                                pb = rowp.tile([P, 512], F32, tag="pbc")
                                nc.tensor.matmul(
                                    pb[:], onesrf_t[:],
                                    rrow_own[0:1, c * 512:(c + 1) * 512],
                                    start=True, stop=True)
                                nc.vector.tensor_copy(
                                    rstd_bc_own[:, c * 512:(c + 1) * 512],
                                    pb[:])
                        for c in range(4):
                            pk = pps.tile([P, 512], F32, tag="pk")
                            for j in range(NJ):
                                nc.tensor.matmul(
                                    pk[:], kwm[:, j * P:(j + 1) * P],
                                    xT_t[j][:, c * 512:(c + 1) * 512],
                                    start=(j == 0), stop=(j == NJ - 1))
                            nc.vector.tensor_copy(
                                kT[m][:, c * 512:(c + 1) * 512], pk[:])
                        pkm = pps.tile([P, T_MEM], F32, tag="pkm", bufs=1)
                        for j in range(NJ):
                            nc.tensor.matmul(
                                pkm[:], kwm[:, j * P:(j + 1) * P],
                                memT_t[:, j * T_MEM:(j + 1) * T_MEM],
                                start=(j == 0), stop=(j == NJ - 1))
                        nc.vector.tensor_copy(kT[m][:, 2048:2048 + T_MEM],
                                              pkm[:])

                # ---- gate (uses xT_lk as lhsT; alpha stored bf16)
                with tc.tile_pool(name="gps", bufs=2, space="PSUM") as gps:
                    for i in range(8):
                        pg = [gps.tile([P, 512], F32, tag=f"pg{oc}",
                                       name=f"pg{i}_{oc}")
                              for oc in range(2)]
                        for j in range(NJ):
                            for oc in range(2):
                                nc.tensor.matmul(
                                    pg[oc][:],
                                    xT_t[j][:, 2 * i * P:(2 * i + 1) * P],
                                    gw_t[j][:, oc * 512:(oc + 1) * 512],
                                    start=(j == 0), stop=False)
                        for oc in range(2):
                            nc.tensor.matmul(
                                pg[oc][:], onesrb_t[:],
                                b_row_t[0:1, oc * 512:(oc + 1) * 512],
                                start=False, stop=True)
                            nc.scalar.activation(
                                alpha[i][:, oc * 512:(oc + 1) * 512],
                                pg[oc][:], AFT.Sigmoid)

            vpool = ctx.enter_context(tc.tile_pool(name="vpool", bufs=1))
            qtp = ctx.enter_context(tc.tile_pool(name="qtp", bufs=1))
            v = [vpool.tile([P, H], BF16, tag=f"v{l}", name=f"v{l}")
                 for l in range(NKV)]
            qT = [qtp.tile([P, 1024], BF16, tag=f"qT{m}", name=f"qT{m}")
                  for m in range(NJ)]

            # ---- qT projection (xT_lk released; own transposed cols)
            with tc.tile_pool(name="wsl2", bufs=2) as wsl2, \
                 tc.tile_pool(name="pps2", bufs=4, space="PSUM") as pps2:
                for l in range(NKV):
                    nc.vector.reciprocal(rstd_nat[:, l:l + 1],
                                         sdt_all[:, l:l + 1])
                nc.vector.tensor_scalar_mul(scl_nat[:], rstd_nat[:], SCALE)
                for m in range(NJ):
                    qwm = wsl2.tile([P, H], BF16, tag="qwm")
                    nc.scalar.dma_start(out=qwm[:],
                                        in_=qw_slab[m * P:(m + 1) * P, :])
                    for c in range(2):
                        pq = pps2.tile([P, 512], F32, tag="pq")
                        for j in range(NJ):
                            nc.tensor.matmul(
                                pq[:], qwm[:, j * P:(j + 1) * P],
                                xo_t[j][:, c * 512:(c + 1) * 512],
                                start=(j == 0), stop=(j == NJ - 1))
                        nc.vector.tensor_mul(
                            qT[m][:, c * 512:(c + 1) * 512], pq[:],
                            rstd_bc_own[:, c * 512:(c + 1) * 512])

            # ---- v pass: v = x * rstd * w  (x re-streamed)
            with tc.tile_pool(name="vps", bufs=3) as vps:
                w_bc_t = const.tile([P, H], BF16)
                nc.sync.dma_start(out=w_bc_t[:], in_=w_bc[:])
                v_mem = const.tile([T_MEM, H], BF16)
                nc.sync.dma_start(out=v_mem[:], in_=mem[:])
                tri2_t = const.tile([P, 256], F32)
                nc.sync.dma_start(out=tri2_t[:], in_=tri2[:])
                nxt2_t = const.tile([P, 256], F32)
                nc.sync.dma_start(out=nxt2_t[:], in_=nxt2[:])
                for l in range(NKV):
                    x2 = vps.tile([P, H], BF16, tag="x2")
                    nc.sync.dma_start(out=x2[:],
                                      in_=x_lk[l * P:(l + 1) * P, :])
                    nc.scalar.activation(v[l][:], x2[:], AFT.Copy,
                                         scale=rstd_nat[:, l:l + 1])
                    nc.vector.tensor_mul(v[l][:], v[l][:], w_bc_t[:])

            if DEBUG:
                nc.sync.dma_start(out=dbg_rstd[:], in_=rstd_nat[:])
                nc.sync.dma_start(
                    out=dbg_bc[:, 0:1024], in_=rstd_bc_own[:])
                nc.sync.dma_start(
                    out=dbg_bc[:, 1024:1040], in_=rstd_nat[:])
                nc.sync.dma_start(out=dbg_al0[:], in_=alpha[0][:])
                nc.sync.dma_start(out=dbg_kt0[:], in_=kT[0][:])
                nc.sync.dma_start(out=dbg_qt0[:], in_=qT[0][:])
                nc.sync.dma_start(out=dbg_v0[:], in_=v[0][:])

            # ================= phase B: attention (4 balanced pairs)
            with tc.tile_pool(name="bst", bufs=2) as bst, \
                 tc.tile_pool(name="bet", bufs=6) as bet, \
                 tc.tile_pool(name="bps", bufs=2, space="PSUM") as bps, \
                 tc.tile_pool(name="bph", bufs=1, space="PSUM") as bph, \
                 tc.tile_pool(name="bpd", bufs=1, space="PSUM") as bpd:
                for p in range(4):
                    q0 = 256 * p
                    xs = []
                    for sl in range(2):
                        xsb = bst.tile([P, H], BF16, tag=f"xsb{sl}")
                        i = 2 * p + sl
                        nc.sync.dma_start(
                            out=xsb[:],
                            in_=x_lk[2 * i * P:(2 * i + 1) * P, :])
                        xst = bst.tile([P, H], F32, tag=f"xs{sl}")
                        nc.vector.tensor_copy(xst[:], xsb[:])
                        xs.append(xst)
                    ph = {(sl, hc): bph.tile([P, 512], F32, tag=f"ph{sl}{hc}",
                                             name=f"ph{sl}{hc}")
                          for sl in range(2) for hc in range(2)}
                    # NB: separate tiles => separate PSUM banks.  start=True
                    # zeroes a whole 2KB zero region, so the two q-blocks'
                    # denominators must not share a bank.
                    pd = [bpd.tile([P, 1], F32, tag=f"pd{sl}", name=f"pd{sl}")
                          for sl in range(2)]

                    # tasks: ("sh", t) both q-blocks; ("so", t) second only; mem
                    tasks = ([("sh", t) for t in range(4 * p + 2)]
                             + [("so", 4 * p + 2), ("so", 4 * p + 3)]
                             + [("mem", 0)])

                    def emit_scores(task):
                        kind, t = task
                        if kind == "sh":
                            ps = bps.tile([P, 256], F32, tag="ps")
                            for m in range(NJ):
                                nc.tensor.matmul(
                                    ps[:], kT[m][:, t * P:(t + 1) * P],
                                    qT[m][:, q0:q0 + 256],
                                    start=(m == 0), stop=(m == NJ - 1))
                            if t == 4 * p:
                                nc.vector.tensor_add(ps[:], ps[:], tri2_t[:])
                            elif t == 4 * p + 1:
                                nc.vector.tensor_add(ps[:], ps[:], nxt2_t[:])
                            et = bet.tile([P, 256], BF16, tag="et")
                            nc.scalar.activation(et[:], ps[:], AFT.Exp,
                                                 scale=scl_nat[:, t:t + 1])
                            return (kind, t, et, P)
                        if kind == "so":
                            ps = bps.tile([P, 256], F32, tag="ps")
                            for m in range(NJ):
                                nc.tensor.matmul(
                                    ps[:, 0:P], kT[m][:, t * P:(t + 1) * P],
                                    qT[m][:, q0 + P:q0 + 256],
                                    start=(m == 0), stop=(m == NJ - 1))
                            bias = tri2_t if t == 4 * p + 2 else nxt2_t
                            nc.vector.tensor_add(ps[:, 0:P], ps[:, 0:P],
                                                 bias[:, 0:P])
                            et = bet.tile([P, 256], BF16, tag="et")
                            nc.scalar.activation(et[:, 0:P], ps[:, 0:P],
                                                 AFT.Exp,
                                                 scale=scl_nat[:, t:t + 1])
                            return (kind, t, et, P)
                        # mem
                        ps = bps.tile([P, 256], F32, tag="ps")
                        for m in range(NJ):
                            nc.tensor.matmul(
                                ps[:T_MEM, :], kT[m][:, 2048:2048 + T_MEM],
                                qT[m][:, q0:q0 + 256],
                                start=(m == 0), stop=(m == NJ - 1))
                        et = bet.tile([P, 256], BF16, tag="et")
                        nc.scalar.activation(et[:T_MEM, :], ps[:T_MEM, :],
                                             AFT.Exp, scale=SCALE)
                        return (kind, t, et, T_MEM)

                    def emit_hv(sc, first, last):
                        kind, t, et, rows = sc
                        if kind == "sh":
                            vt, sls = v[t], (0, 1)
                        elif kind == "so":
                            vt, sls = v[t], (1,)
                        else:
                            vt, sls = v_mem, (0, 1)
                        for sl in sls:
                            if kind == "so":
                                lhs = et[:rows, 0:P]
                            else:
                                lhs = et[:rows, sl * P:(sl + 1) * P]
                            for hc in range(2):
                                nc.tensor.matmul(
                                    ph[(sl, hc)][:], lhs,
                                    vt[:rows, hc * 512:(hc + 1) * 512],
                                    start=first, stop=last,
                                    skip_group_check=True)
                            nc.tensor.matmul(
                                pd[sl][:], lhs, onesc_t[:rows, :],
                                start=first, stop=last,
                                skip_group_check=True)

                    scs = []
                    for ti, task in enumerate(tasks):
                        sc = emit_scores(task)
                        if DEBUG and p == 0 and ti == 0:
                            nc.sync.dma_start(out=dbg_et0[:], in_=sc[2][:])
                        if DEBUG and p == 0 and task[0] == "mem":
                            nc.sync.dma_start(out=dbg_etm[:], in_=sc[2][:])
                        scs.append(sc)
                        if len(scs) > 2:
                            old_sc = scs.pop(0)
                            emit_hv(old_sc,
                                    first=(old_sc[1] == 0 and old_sc[0] == "sh"),
                                    last=False)
                    while scs:
                        old_sc = scs.pop(0)
                        emit_hv(old_sc,
                                first=(old_sc[1] == 0 and old_sc[0] == "sh"),
                                last=(not scs))
                    if DEBUG and p == 0:
                        dsb = bst.tile([P, 2], F32, tag="dsb", name="dsb")
                        nc.vector.tensor_copy(dsb[:, 0:1], pd[0][:])
                        nc.vector.tensor_copy(dsb[:, 1:2], pd[1][:])
                        nc.sync.dma_start(out=dbg_den[:], in_=dsb[:])
                        hsb0 = bst.tile([P, 512], F32, tag="hsb0", name="hsb0")
                        nc.vector.tensor_copy(hsb0[:], ph[(0, 0)][:])
                        nc.sync.dma_start(out=dbg_h00[:], in_=hsb0[:])

                    # evict pair
                    for sl in range(2):
                        i = 2 * p + sl
                        rd = bst.tile([P, 1], F32, tag="rd")
                        nc.vector.reciprocal(rd[:], pd[sl][:])
                        hsb = bst.tile([P, H], F32, tag="hsb")
                        for hc in range(2):
                            nc.vector.tensor_scalar_mul(
                                hsb[:, hc * 512:(hc + 1) * 512],
                                ph[(sl, hc)][:], rd[:])
                        nc.vector.tensor_sub(hsb[:], hsb[:], xs[sl][:])
                        nc.vector.tensor_mul(hsb[:], hsb[:], alpha[i][:])
                        ho = bst.tile([P, H], BF16, tag="ho")
                        nc.vector.tensor_add(ho[:], hsb[:], xs[sl][:])
                        nc.sync.dma_start(out=out[i * P:(i + 1) * P, :],
                                          in_=ho[:])

    import os
    if os.environ.get("NO_WAIT_SPLIT") != "1":
        split_multi_waits(nc, limit=1, dma_limit=1)
    return nc


_NC_CACHE = None
_LAST_IN_MAPS = None


def _get_nc():
    global _NC_CACHE
    if _NC_CACHE is None:
        _NC_CACHE = build_nc()
    return _NC_CACHE


def _slab(A):
    """[o,h] weight -> per-m stationary slabs: slab[128m+p, 128j+c] = A[128m+c, 128j+p]."""
    B4 = A.reshape(8, P, 8, P)                 # [m, c, j, p]
    return np.ascontiguousarray(
        B4.transpose(0, 3, 2, 1).reshape(H, H)).astype(NPBF16)


def prepare_in_maps(hidden_states, memory_state, q_w, k_w, norm_w, gate_w,
                    gate_b):
    hidden_states = np.asarray(hidden_states, dtype=np.float32)
    memory_state = np.asarray(memory_state, dtype=np.float32)
    q_w = np.asarray(q_w, dtype=np.float32)
    k_w = np.asarray(k_w, dtype=np.float32)
    norm_w = np.asarray(norm_w, dtype=np.float32)
    gate_w = np.asarray(gate_w, dtype=np.float32)
    gate_b = np.asarray(gate_b, dtype=np.float32)

    qw_slab = _slab(q_w * norm_w[None, :])
    kw_slab = _slab(k_w * norm_w[None, :])
    gw = np.ascontiguousarray(gate_w.T).astype(NPBF16)
    b_row = np.ascontiguousarray(gate_b[None, :]).astype(NPBF16)
    w_bc = np.ascontiguousarray(
        np.broadcast_to(norm_w, (P, H))).astype(NPBF16)
    tri = np.where(np.arange(P)[None, :] >= np.arange(P)[:, None],
                   np.float32(0.0), np.float32(NEG)).astype(np.float32)
    tri2 = np.concatenate([tri, np.zeros((P, P), np.float32)], axis=1)

    in_maps = []
    for c in range(N_CORES):
        b, h = divmod(c, 2)
        xb = hidden_states[b]                       # [2048, H]
        blocks = xb.reshape(NKV, P, H)
        lk_order = []
        for i in range(8):
            lk_order += [2 * i + h, 2 * i + (1 - h)]
        x_lk = blocks[lk_order].reshape(2048, H)
        own = blocks[[2 * i + h for i in range(8)]].reshape(1024, H)
        memb = memory_state[b]                      # [64, H]
        memT_slab = np.ascontiguousarray(
            memb.reshape(T_MEM, 8, P).transpose(2, 1, 0).reshape(P, 512)
        ).astype(NPBF16)
        nxt_col = np.full((P, P), NEG if h == 0 else 0.0, np.float32)
        nxt2 = np.concatenate([nxt_col, np.zeros((P, P), np.float32)], axis=1)
        in_maps.append({
            "x_lk": x_lk.astype(NPBF16),
            "xT_lk": np.ascontiguousarray(x_lk.T).astype(NPBF16),
            "xT_own": np.ascontiguousarray(own.T).astype(NPBF16),
            "mem": memb.astype(NPBF16),
            "memT_slab": memT_slab,
            "qw_slab": qw_slab, "kw_slab": kw_slab, "gw": gw,
            "b_row": b_row, "w_bc": w_bc,
            "tri2": tri2, "nxt2": nxt2,
            "onesc": np.ones((P, 1), NPBF16),
            "onesr_b": np.ones((1, P), NPBF16),
            "onesr_f": np.ones((1, P), np.float32),
        })
    return in_maps


def kernel(**inputs):
    in_maps = prepare_in_maps(**inputs)
    global _LAST_IN_MAPS
    _LAST_IN_MAPS = in_maps
    nc = _get_nc()
    res = run_bass_kernel_spmd(nc, in_maps, list(range(N_CORES)))
    out = np.empty((B_FULL, S_FULL, H), dtype=np.float32)
    for c in range(N_CORES):
        b, h = divmod(c, 2)
        r = res.results[c]["out"].astype(np.float32)
        for i in range(8):
            g = 2 * i + h
            out[b, g * P:(g + 1) * P] = r[i * P:(i + 1) * P]
    return out


# revision 26
# speedup vs baseline: 1.7933x; 1.0008x over previous
"""Trainium2 Bass kernel for nn_MemResProjections (memory-residual attention).

Reference computation (B=4, S=2048, K=64, H=1024, fp32):
    normed = rmsnorm(hidden) * norm_w
    v_pool = concat([normed, memory], axis=1)            # (B, S+K, H)
    q = normed @ q_w.T ; k = v_pool @ k_w.T
    logits = q @ k.T / sqrt(H)  with causal mask on the local S block,
    memory columns fully visible
    attn = softmax(logits); h~ = attn @ v_pool
    alpha = sigmoid(hidden @ gate_w.T + gate_b)
    out = (1-alpha)*hidden + alpha*h~

Sharding: 8 cores = (batch b, parity h) pairs.  Core (b,h) owns the 8
query blocks {h, h+2, ..., h+14} (128 rows each) of batch b.  KV blocks are
laid out per-core in a "local" interleaved order  [own_0, oth_0, own_1,
oth_1, ...]  so that own query block i needs exactly the kv-local prefix
0..2i+1 on every core -- one uniform program, per-core data.  The single
per-core mask difference (is the interleaved neighbour before or after me?)
is a data tile (nxt2: -1e30 for h=0, 0 for h=1).

All activations/weights move as bf16 (host-prepped, incl. transposed copies
of x so no PE transposes are needed); psum accumulation fp32; rmsnorm stats
on-device in fp32.  No DRAM spills: v, kT, qT, alpha stay SBUF-resident.
rstd is folded in at PSUM eviction via a partition-broadcast tile built
with a tiny ones-matmul.  Phase order keeps PE hot: stats (ACT) -> kT ->
gate -> qT -> v scaling -> attention; weight/xT loads ride the scalar
HWDGE ring so the sync ring serves the x stripes immediately.
"""
import numpy as np
import ml_dtypes

import concourse.bass as bass
import concourse.mybir as mybir
import concourse.tile as tile
from concourse.bass_utils import run_bass_kernel_spmd

BF16 = mybir.dt.bfloat16
F32 = mybir.dt.float32
F32R = mybir.dt.float32r
AFT = mybir.ActivationFunctionType
NPBF16 = ml_dtypes.bfloat16

P = 128
H = 1024
NJ = H // P           # 8 h-blocks
NKV = 16              # local kv blocks (2048 rows)
T_MEM = 64
SCALE = 1.0 / 32.0    # 1/sqrt(H)
EPS = 1e-6
NEG = -1.0e30

N_CORES = 8
B_FULL, S_FULL = 4, 2048


# ---------------------------------------------------------------- walrus fix
ENGINE_ATTR = {
    mybir.EngineType.PE: "tensor",
    mybir.EngineType.Activation: "scalar",
    mybir.EngineType.DVE: "vector",
    mybir.EngineType.Pool: "gpsimd",
    mybir.EngineType.SP: "sync",
}
DMA_OPS = ("InstDMACopy", "InstDMATranspose", "InstTensorLoad", "InstTensorSave",
           "InstCollectiveCompute")


def split_multi_waits(nc, limit=1, dma_limit=None):
    """This walrus build rejects engine instructions carrying more than one
    sem wait; hoist extras onto same-engine NOPs inserted just before."""
    n_split = 0
    for f in nc.m.functions:
        for blk in f.blocks:
            il = blk.instructions
            i = 0
            while i < len(il):
                ins = il[i]
                is_dma = type(ins).__name__ in DMA_OPS
                lim = dma_limit if is_dma else limit
                si = ins.sync_info
                waits = list(si.on_wait) if si is not None and si.on_wait else []
                if lim is not None and len(waits) > lim:
                    keep, extra = waits[:lim], waits[lim:]
                    si.on_wait.clear()
                    for w in keep:
                        si.on_wait.append(w)
                    eng = getattr(nc, ENGINE_ATTR[ins.engine])
                    for w in extra:
                        nop = eng.nop(nofuse=True, hint="wait_split")
                        nop.wait_op(bass.SemaphoreHandle(w.ant_name, w.id),
                                    w.wait_value, "sem-ge")
                        popped = nc.cur_bb.bb.instructions.pop()
                        assert popped.name == nop.ins.name
                        il.insert(i, nop.ins)
                        i += 1
                        n_split += 1
                i += 1
    return n_split


# ---------------------------------------------------------------- program
def build_nc():
    nc = bass.Bass()
    dp = lambda n, shp, dt: nc.declare_dram_parameter(n, shp, dt, isOutput=False)
    x_lk = dp("x_lk", [2048, H], BF16)        # natural, local-kv row order
    xT_lk = dp("xT_lk", [H, 2048], BF16)      # transposed, same col order
    xT_own = dp("xT_own", [H, 1024], BF16)    # own columns only
    mem = dp("mem", [T_MEM, H], BF16)
    memT_slab = dp("memT_slab", [P, 512], BF16)   # [p, 64j+t] = mem[t, 128j+p]
    qw_slab = dp("qw_slab", [H, H], BF16)     # slab m rows: [p, 128j+c] = qwT_w[128j+p, 128m+c]
    kw_slab = dp("kw_slab", [H, H], BF16)
    gw = dp("gw", [H, H], BF16)               # gate_w.T  [h, o]
    b_row = dp("b_row", [1, H], BF16)         # gate_b
    w_bc = dp("w_bc", [P, H], BF16)           # norm_w broadcast
    tri2 = dp("tri2", [P, 256], F32)          # [tri | 0]
    nxt2 = dp("nxt2", [P, 256], F32)          # [nextb | 0], nextb = -1e30 (h=0) / 0 (h=1)
    onesc = dp("onesc", [P, 1], BF16)
    onesr_b = dp("onesr_b", [1, P], BF16)
    onesr_f = dp("onesr_f", [1, P], F32)
    out = nc.declare_dram_parameter("out", [1024, H], BF16, isOutput=True)
    import os as _os
    DEBUG = _os.environ.get("DEBUG_KERNEL") == "1"
    if DEBUG:
        dout = lambda n, shp: nc.declare_dram_parameter(n, shp, F32,
                                                        isOutput=True)
        dbg_rstd = dout("dbg_rstd", [P, NKV])
        dbg_bc = dout("dbg_bc", [P, 2048])
        dbg_al0 = dout("dbg_al0", [P, H])
        dbg_kt0 = nc.declare_dram_parameter("dbg_kt0", [P, 2048 + T_MEM],
                                            BF16, isOutput=True)
        dbg_qt0 = nc.declare_dram_parameter("dbg_qt0", [P, 1024], BF16,
                                            isOutput=True)
        dbg_v0 = nc.declare_dram_parameter("dbg_v0", [P, H], BF16,
                                           isOutput=True)
        dbg_et0 = nc.declare_dram_parameter("dbg_et0", [P, 256], BF16,
                                            isOutput=True)
        dbg_etm = nc.declare_dram_parameter("dbg_etm", [P, 256], BF16,
                                            isOutput=True)
        dbg_den = dout("dbg_den", [P, 2])
        dbg_h00 = dout("dbg_h00", [P, 512])

    with tile.TileContext(nc) as tc:
        from contextlib import ExitStack
        with ExitStack() as ctx:
            # ---- long-lived pools (v/qT open after phase A to keep the
            # peak under the SBUF cap -- pools reserve for their whole scope)
            const = ctx.enter_context(tc.tile_pool(name="const", bufs=1))
            ktp = ctx.enter_context(tc.tile_pool(name="ktp", bufs=1))
            apool = ctx.enter_context(tc.tile_pool(name="apool", bufs=1))

            eps_t = const.tile([P, 1], F32)
            nc.vector.memset(eps_t[:], EPS)
            rstd_nat = const.tile([P, NKV], F32)
            sdt_all = const.tile([P, NKV], F32)
            scl_nat = const.tile([P, NKV], F32)
            rrow_own = const.tile([1, 1024], F32R)
            std_row = const.tile([1, 1024], F32)
            rrow_f = const.tile([1, 1024], F32)
            rstd_bc_own = const.tile([P, 1024], F32)

            kT = [ktp.tile([P, 2048 + T_MEM], BF16, tag=f"kT{m}", name=f"kT{m}")
                  for m in range(NJ)]
            alpha = [apool.tile([P, H], F32, tag=f"al{i}", name=f"al{i}")
                     for i in range(8)]

            # xT_lk tiles live through kT + gate.  Only the 8 xT loads ride
            # the scalar HWDGE ring (so ACT is free after ~5us); everything
            # else is issued on the sync ring in criticality order.
            xop = ctx.enter_context(tc.tile_pool(name="xop", bufs=1))
            with tc.tile_pool(name="xtpool", bufs=1) as xtpool, \
                 tc.tile_pool(name="sqo", bufs=1) as sqo, \
                 tc.tile_pool(name="kwp", bufs=1) as kwp, \
                 tc.tile_pool(name="gwp", bufs=1) as gwp:
                xT_t = [xtpool.tile([P, 2048], BF16, tag=f"xT{j}", name=f"xT{j}")
                        for j in range(NJ)]
                for j in range(NJ):
                    nc.sync.dma_start(out=xT_t[j][:],
                                      in_=xT_lk[j * P:(j + 1) * P, :])
                kw_t = [kwp.tile([P, H], BF16, tag=f"kw{m}", name=f"kw{m}")
                        for m in range(NJ)]
                for m in range(NJ):
                    nc.scalar.dma_start(out=kw_t[m][:],
                                        in_=kw_slab[m * P:(m + 1) * P, :])
                memT_t = const.tile([P, 512], BF16)
                nc.scalar.dma_start(out=memT_t[:], in_=memT_slab[:])
                onesc_t = const.tile([P, 1], BF16)
                nc.scalar.dma_start(out=onesc_t[:], in_=onesc[:])
                onesrf_t = const.tile([1, P], F32R)
                nc.scalar.dma_start(out=onesrf_t[:], in_=onesr_f[:].bitcast(F32R))

                # ---- A1: rmsnorm stats on ACT only (reciprocals deferred so
                # the DVE stream is free for kT evictions)
                with tc.tile_pool(name="a1s", bufs=3) as a1s:
                    for l in range(NKV):
                        xt = a1s.tile([P, H], BF16, tag="xt")
                        nc.sync.dma_start(out=xt[:],
                                          in_=x_lk[l * P:(l + 1) * P, :])
                        sq = a1s.tile([P, H], BF16, tag="sq", bufs=2)
                        ss = a1s.tile([P, 1], F32, tag="ss")
                        nc.scalar.activation(sq[:], xt[:], AFT.Square,
                                             accum_out=ss[:])
                        nc.scalar.activation(sdt_all[:, l:l + 1], ss[:],
                                             AFT.Sqrt, scale=1.0 / H,
                                             bias=eps_t[:])
                xo_t = [xop.tile([P, 1024], BF16, tag=f"xo{j}", name=f"xo{j}")
                        for j in range(NJ)]
                for j in range(NJ):
                    nc.sync.dma_start(out=xo_t[j][:],
                                      in_=xT_own[j * P:(j + 1) * P, :])
                gw_t = [gwp.tile([P, H], BF16, tag=f"gw{j}", name=f"gw{j}")
                        for j in range(NJ)]
                for j in range(NJ):
                    nc.sync.dma_start(out=gw_t[j][:],
                                      in_=gw[j * P:(j + 1) * P, :])
                onesrb_t = const.tile([1, P], BF16)
                nc.sync.dma_start(out=onesrb_t[:], in_=onesr_b[:])
                b_row_t = const.tile([1, H], BF16)
                nc.sync.dma_start(out=b_row_t[:], in_=b_row[:])
                # squares of own transposed cols for the row-form stats
                sq_own = [sqo.tile([P, 1024], BF16, tag=f"sqo{j}", name=f"sqo{j}")
                          for j in range(NJ)]
                for j in range(NJ):
                    nc.scalar.activation(sq_own[j][:], xo_t[j][:],
                                         AFT.Square)

                # ---- kT projection (row-form q-rstd stats interleaved so
                # rstd_bc_own is ready long before the qT evictions)
                with tc.tile_pool(name="rowp", bufs=1, space="PSUM") as rowp, \
                     tc.tile_pool(name="pps", bufs=4, space="PSUM") as pps:
                    for m in range(NJ):
                        kwm = kw_t[m]
                        if m == 3:
                            for c in range(2):
                                pv = rowp.tile([1, 512], F32, tag="pv")
                                for j in range(NJ):
                                    nc.tensor.matmul(
                                        pv[:], onesc_t[:],
                                        sq_own[j][:, c * 512:(c + 1) * 512],
                                        start=(j == 0), stop=(j == NJ - 1))
                                nc.scalar.activation(
                                    std_row[0:1, c * 512:(c + 1) * 512],
                                    pv[:], AFT.Sqrt, scale=1.0 / H,
                                    bias=eps_t[0:1, :])
                            nc.vector.reciprocal(rrow_f[:], std_row[:])
                            nc.scalar.activation(rrow_own[:], rrow_f[:],
                                                 AFT.Copy)
                        if m == 4:
                            for c in range(2):
                                pb = rowp.tile([P, 512], F32, tag="pbc")
                                nc.tensor.matmul(
                                    pb[:], onesrf_t[:],
                                    rrow_own[0:1, c * 512:(c + 1) * 512],
                                    start=True, stop=True)
                                nc.vector.tensor_copy(
                                    rstd_bc_own[:, c * 512:(c + 1) * 512],
                                    pb[:])
                        for c in range(4):
                            pk = pps.tile([P, 512], F32, tag="pk")
                            for j in range(NJ):
                                nc.tensor.matmul(
                                    pk[:], kwm[:, j * P:(j + 1) * P],
                                    xT_t[j][:, c * 512:(c + 1) * 512],
                                    start=(j == 0), stop=(j == NJ - 1))
                            nc.vector.tensor_copy(
                                kT[m][:, c * 512:(c + 1) * 512], pk[:])
                        pkm = pps.tile([P, T_MEM], F32, tag="pkm", bufs=1)
                        for j in range(NJ):
                            nc.tensor.matmul(
                                pkm[:], kwm[:, j * P:(j + 1) * P],
                                memT_t[:, j * T_MEM:(j + 1) * T_MEM],
                                start=(j == 0), stop=(j == NJ - 1))
                        nc.vector.tensor_copy(kT[m][:, 2048:2048 + T_MEM],
                                              pkm[:])

                # ---- gate (uses xT_lk as lhsT; alpha stored bf16)
                with tc.tile_pool(name="gps", bufs=2, space="PSUM") as gps:
                    for i in range(8):
                        pg = [gps.tile([P, 512], F32, tag=f"pg{oc}",
                                       name=f"pg{i}_{oc}")
                              for oc in range(2)]
                        for j in range(NJ):
                            for oc in range(2):
                                nc.tensor.matmul(
                                    pg[oc][:],
                                    xT_t[j][:, 2 * i * P:(2 * i + 1) * P],
                                    gw_t[j][:, oc * 512:(oc + 1) * 512],
                                    start=(j == 0), stop=False)
                        for oc in range(2):
                            nc.tensor.matmul(
                                pg[oc][:], onesrb_t[:],
                                b_row_t[0:1, oc * 512:(oc + 1) * 512],
                                start=False, stop=True)
                            nc.scalar.activation(
                                alpha[i][:, oc * 512:(oc + 1) * 512],
                                pg[oc][:], AFT.Sigmoid)

            vpool = ctx.enter_context(tc.tile_pool(name="vpool", bufs=1))
            qtp = ctx.enter_context(tc.tile_pool(name="qtp", bufs=1))
            v = [vpool.tile([P, H], BF16, tag=f"v{l}", name=f"v{l}")
                 for l in range(NKV)]
            qT = [qtp.tile([P, 1024], BF16, tag=f"qT{m}", name=f"qT{m}")
                  for m in range(NJ)]

            # ---- qT projection (xT_lk released; own transposed cols)
            with tc.tile_pool(name="wsl2", bufs=2) as wsl2, \
                 tc.tile_pool(name="pps2", bufs=4, space="PSUM") as pps2:
                for l in range(NKV):
                    nc.vector.reciprocal(rstd_nat[:, l:l + 1],
                                         sdt_all[:, l:l + 1])
                nc.vector.tensor_scalar_mul(scl_nat[:], rstd_nat[:], SCALE)
                for m in range(NJ):
                    qwm = wsl2.tile([P, H], BF16, tag="qwm")
                    nc.scalar.dma_start(out=qwm[:],
                                        in_=qw_slab[m * P:(m + 1) * P, :])
                    for c in range(2):
                        pq = pps2.tile([P, 512], F32, tag="pq")
                        for j in range(NJ):
                            nc.tensor.matmul(
                                pq[:], qwm[:, j * P:(j + 1) * P],
                                xo_t[j][:, c * 512:(c + 1) * 512],
                                start=(j == 0), stop=(j == NJ - 1))
                        nc.vector.tensor_mul(
                            qT[m][:, c * 512:(c + 1) * 512], pq[:],
                            rstd_bc_own[:, c * 512:(c + 1) * 512])

            # ---- v pass: v = x * rstd * w  (x re-streamed)
            with tc.tile_pool(name="vps", bufs=3) as vps:
                w_bc_t = const.tile([P, H], BF16)
                nc.sync.dma_start(out=w_bc_t[:], in_=w_bc[:])
                v_mem = const.tile([T_MEM, H], BF16)
                nc.sync.dma_start(out=v_mem[:], in_=mem[:])
                tri2_t = const.tile([P, 256], F32)
                nc.sync.dma_start(out=tri2_t[:], in_=tri2[:])
                nxt2_t = const.tile([P, 256], F32)
                nc.sync.dma_start(out=nxt2_t[:], in_=nxt2[:])
                for l in range(NKV):
                    x2 = vps.tile([P, H], BF16, tag="x2")
                    nc.sync.dma_start(out=x2[:],
                                      in_=x_lk[l * P:(l + 1) * P, :])
                    nc.scalar.activation(v[l][:], x2[:], AFT.Copy,
                                         scale=rstd_nat[:, l:l + 1])
                    nc.vector.tensor_mul(v[l][:], v[l][:], w_bc_t[:])

            if DEBUG:
                nc.sync.dma_start(out=dbg_rstd[:], in_=rstd_nat[:])
                nc.sync.dma_start(
                    out=dbg_bc[:, 0:1024], in_=rstd_bc_own[:])
                nc.sync.dma_start(
                    out=dbg_bc[:, 1024:1040], in_=rstd_nat[:])
                nc.sync.dma_start(out=dbg_al0[:], in_=alpha[0][:])
                nc.sync.dma_start(out=dbg_kt0[:], in_=kT[0][:])
                nc.sync.dma_start(out=dbg_qt0[:], in_=qT[0][:])
                nc.sync.dma_start(out=dbg_v0[:], in_=v[0][:])

            # ================= phase B: attention (4 balanced pairs)
            with tc.tile_pool(name="bst", bufs=2) as bst, \
                 tc.tile_pool(name="bet", bufs=6) as bet, \
                 tc.tile_pool(name="bps", bufs=3, space="PSUM") as bps, \
                 tc.tile_pool(name="bph", bufs=1, space="PSUM") as bph, \
                 tc.tile_pool(name="bpd", bufs=1, space="PSUM") as bpd:
                for p in range(4):
                    q0 = 256 * p
                    xs = []
                    for sl in range(2):
                        xsb = bst.tile([P, H], BF16, tag=f"xsb{sl}")
                        i = 2 * p + sl
                        nc.sync.dma_start(
                            out=xsb[:],
                            in_=x_lk[2 * i * P:(2 * i + 1) * P, :])
                        xst = bst.tile([P, H], F32, tag=f"xs{sl}")
                        nc.vector.tensor_copy(xst[:], xsb[:])
                        xs.append(xst)
                    ph = {(sl, hc): bph.tile([P, 512], F32, tag=f"ph{sl}{hc}",
                                             name=f"ph{sl}{hc}")
                          for sl in range(2) for hc in range(2)}
                    # One [P,2] tile: sl0's first MM carries the only
                    # start=True, zeroing the whole 2KB bank (both columns);
                    # sl1's first MM rides the cleared has_written bits with
                    # start=False (first write per element overwrites).
                    pdt = bpd.tile([P, 2], F32, tag="pd", name="pd")
                    pd = [pdt[:, 0:1], pdt[:, 1:2]]

                    # tasks: ("sh", t) both q-blocks; ("so", t) second only; mem
                    tasks = ([("sh", t) for t in range(4 * p + 2)]
                             + [("so", 4 * p + 2), ("so", 4 * p + 3)]
                             + [("mem", 0)])

                    def emit_scores(task):
                        kind, t = task
                        if kind == "sh":
                            ps = bps.tile([P, 256], F32, tag="ps")
                            for m in range(NJ):
                                nc.tensor.matmul(
                                    ps[:], kT[m][:, t * P:(t + 1) * P],
                                    qT[m][:, q0:q0 + 256],
                                    start=(m == 0), stop=(m == NJ - 1))
                            if t == 4 * p:
                                nc.vector.tensor_add(ps[:], ps[:], tri2_t[:])
                            elif t == 4 * p + 1:
                                nc.vector.tensor_add(ps[:], ps[:], nxt2_t[:])
                            et = bet.tile([P, 256], BF16, tag="et")
                            nc.scalar.activation(et[:], ps[:], AFT.Exp,
                                                 scale=scl_nat[:, t:t + 1])
                            return (kind, t, et, P)
                        if kind == "so":
                            ps = bps.tile([P, 256], F32, tag="ps")
                            for m in range(NJ):
                                nc.tensor.matmul(
                                    ps[:, 0:P], kT[m][:, t * P:(t + 1) * P],
                                    qT[m][:, q0 + P:q0 + 256],
                                    start=(m == 0), stop=(m == NJ - 1))
                            bias = tri2_t if t == 4 * p + 2 else nxt2_t
                            nc.vector.tensor_add(ps[:, 0:P], ps[:, 0:P],
                                                 bias[:, 0:P])
                            et = bet.tile([P, 256], BF16, tag="et")
                            nc.scalar.activation(et[:, 0:P], ps[:, 0:P],
                                                 AFT.Exp,
                                                 scale=scl_nat[:, t:t + 1])
                            return (kind, t, et, P)
                        # mem
                        ps = bps.tile([P, 256], F32, tag="ps")
                        for m in range(NJ):
                            nc.tensor.matmul(
                                ps[:T_MEM, :], kT[m][:, 2048:2048 + T_MEM],
                                qT[m][:, q0:q0 + 256],
                                start=(m == 0), stop=(m == NJ - 1))
                        et = bet.tile([P, 256], BF16, tag="et")
                        nc.scalar.activation(et[:T_MEM, :], ps[:T_MEM, :],
                                             AFT.Exp, scale=SCALE)
                        return (kind, t, et, T_MEM)

                    def emit_hv(sc, first, last):
                        kind, t, et, rows = sc
                        if kind == "sh":
                            vt, sls = v[t], (0, 1)
                        elif kind == "so":
                            vt, sls = v[t], (1,)
                        else:
                            vt, sls = v_mem, (0, 1)
                        for sl in sls:
                            if kind == "so":
                                lhs = et[:rows, 0:P]
                            else:
                                lhs = et[:rows, sl * P:(sl + 1) * P]
                            for hc in range(2):
                                nc.tensor.matmul(
                                    ph[(sl, hc)][:], lhs,
                                    vt[:rows, hc * 512:(hc + 1) * 512],
                                    start=first, stop=last,
                                    skip_group_check=True)
                            nc.tensor.matmul(
                                pd[sl], lhs, onesc_t[:rows, :],
                                start=(first and sl == 0), stop=last,
                                skip_group_check=True)

                    scs = []
                    for ti, task in enumerate(tasks):
                        sc = emit_scores(task)
                        if DEBUG and p == 0 and ti == 0:
                            nc.sync.dma_start(out=dbg_et0[:], in_=sc[2][:])
                        if DEBUG and p == 0 and task[0] == "mem":
                            nc.sync.dma_start(out=dbg_etm[:], in_=sc[2][:])
                        scs.append(sc)
                        if len(scs) > 2:
                            old_sc = scs.pop(0)
                            emit_hv(old_sc,
                                    first=(old_sc[1] == 0 and old_sc[0] == "sh"),
                                    last=False)
                    while scs:
                        old_sc = scs.pop(0)
                        emit_hv(old_sc,
                                first=(old_sc[1] == 0 and old_sc[0] == "sh"),
                                last=(not scs))
                    if DEBUG and p == 0:
                        dsb = bst.tile([P, 2], F32, tag="dsb", name="dsb")
                        nc.vector.tensor_copy(dsb[:], pdt[:])
                        nc.sync.dma_start(out=dbg_den[:], in_=dsb[:])
                        hsb0 = bst.tile([P, 512], F32, tag="hsb0", name="hsb0")
                        nc.vector.tensor_copy(hsb0[:], ph[(0, 0)][:])
                        nc.sync.dma_start(out=dbg_h00[:], in_=hsb0[:])

                    # evict pair
                    for sl in range(2):
                        i = 2 * p + sl
                        rd = bst.tile([P, 1], F32, tag="rd")
                        nc.vector.reciprocal(rd[:], pd[sl])
                        hsb = bst.tile([P, H], F32, tag="hsb")
                        for hc in range(2):
                            nc.vector.tensor_scalar_mul(
                                hsb[:, hc * 512:(hc + 1) * 512],
                                ph[(sl, hc)][:], rd[:])
                        nc.vector.tensor_sub(hsb[:], hsb[:], xs[sl][:])
                        nc.vector.tensor_mul(hsb[:], hsb[:], alpha[i][:])
                        ho = bst.tile([P, H], BF16, tag="ho")
                        nc.vector.tensor_add(ho[:], hsb[:], xs[sl][:])
                        nc.sync.dma_start(out=out[i * P:(i + 1) * P, :],
                                          in_=ho[:])

    import os
    if os.environ.get("NO_WAIT_SPLIT") != "1":
        split_multi_waits(nc, limit=1, dma_limit=1)
    return nc


_NC_CACHE = None
_LAST_IN_MAPS = None


def _get_nc():
    global _NC_CACHE
    if _NC_CACHE is None:
        _NC_CACHE = build_nc()
    return _NC_CACHE


def _slab(A):
    """[o,h] weight -> per-m stationary slabs: slab[128m+p, 128j+c] = A[128m+c, 128j+p]."""
    B4 = A.reshape(8, P, 8, P)                 # [m, c, j, p]
    return np.ascontiguousarray(
        B4.transpose(0, 3, 2, 1).reshape(H, H)).astype(NPBF16)


def prepare_in_maps(hidden_states, memory_state, q_w, k_w, norm_w, gate_w,
                    gate_b):
    hidden_states = np.asarray(hidden_states, dtype=np.float32)
    memory_state = np.asarray(memory_state, dtype=np.float32)
    q_w = np.asarray(q_w, dtype=np.float32)
    k_w = np.asarray(k_w, dtype=np.float32)
    norm_w = np.asarray(norm_w, dtype=np.float32)
    gate_w = np.asarray(gate_w, dtype=np.float32)
    gate_b = np.asarray(gate_b, dtype=np.float32)

    qw_slab = _slab(q_w * norm_w[None, :])
    kw_slab = _slab(k_w * norm_w[None, :])
    gw = np.ascontiguousarray(gate_w.T).astype(NPBF16)
    b_row = np.ascontiguousarray(gate_b[None, :]).astype(NPBF16)
    w_bc = np.ascontiguousarray(
        np.broadcast_to(norm_w, (P, H))).astype(NPBF16)
    tri = np.where(np.arange(P)[None, :] >= np.arange(P)[:, None],
                   np.float32(0.0), np.float32(NEG)).astype(np.float32)
    tri2 = np.concatenate([tri, np.zeros((P, P), np.float32)], axis=1)

    in_maps = []
    for c in range(N_CORES):
        b, h = divmod(c, 2)
        xb = hidden_states[b]                       # [2048, H]
        blocks = xb.reshape(NKV, P, H)
        lk_order = []
        for i in range(8):
            lk_order += [2 * i + h, 2 * i + (1 - h)]
        x_lk = blocks[lk_order].reshape(2048, H)
        own = blocks[[2 * i + h for i in range(8)]].reshape(1024, H)
        memb = memory_state[b]                      # [64, H]
        memT_slab = np.ascontiguousarray(
            memb.reshape(T_MEM, 8, P).transpose(2, 1, 0).reshape(P, 512)
        ).astype(NPBF16)
        nxt_col = np.full((P, P), NEG if h == 0 else 0.0, np.float32)
        nxt2 = np.concatenate([nxt_col, np.zeros((P, P), np.float32)], axis=1)
        in_maps.append({
            "x_lk": x_lk.astype(NPBF16),
            "xT_lk": np.ascontiguousarray(x_lk.T).astype(NPBF16),
            "xT_own": np.ascontiguousarray(own.T).astype(NPBF16),
            "mem": memb.astype(NPBF16),
            "memT_slab": memT_slab,
            "qw_slab": qw_slab, "kw_slab": kw_slab, "gw": gw,
            "b_row": b_row, "w_bc": w_bc,
            "tri2": tri2, "nxt2": nxt2,
            "onesc": np.ones((P, 1), NPBF16),
            "onesr_b": np.ones((1, P), NPBF16),
            "onesr_f": np.ones((1, P), np.float32),
        })
    return in_maps


def kernel(**inputs):
    in_maps = prepare_in_maps(**inputs)
    global _LAST_IN_MAPS
    _LAST_IN_MAPS = in_maps
    nc = _get_nc()
    res = run_bass_kernel_spmd(nc, in_maps, list(range(N_CORES)))
    out = np.empty((B_FULL, S_FULL, H), dtype=np.float32)
    for c in range(N_CORES):
        b, h = divmod(c, 2)
        r = res.results[c]["out"].astype(np.float32)
        for i in range(8):
            g = 2 * i + h
            out[b, g * P:(g + 1) * P] = r[i * P:(i + 1) * P]
    return out
